# revision 1
# baseline (speedup 1.0000x reference)
"""Trainium2 Bass kernel for local windowed MHA (nn_LocalMHA).

Computation (see reference): x (C=1024, T=16384) -> LayerNorm over C ->
QKV proj -> rotary (window-relative) -> per-head attention within windows
of 32 tokens -> out proj -> +x residual.

Sharding: T split across 8 cores (2048 tokens each); windows are local so
no communication is needed. Weights replicated.

Layout strategy (per core, per 512-token chunk):
  - x stays channel-major (C on partitions) so it feeds matmuls directly.
  - LN stats via ones-matmul on the tensor engine (fp32r, full rate at
    N=512); per-column mean/rstd broadcast across partitions via K=1
    matmuls; normalize with 2 DVE ops per 128-row tile.
  - q^T, k^T computed channel-major (pre-transposed weights as lhsT,
    normed as rhs); v computed token-major (normed as lhsT, weights as
    rhs).  gamma is folded into the weights host-side; beta becomes a
    per-output-channel bias applied during PSUM eviction.
  - rotary applied on eviction: q_rot = q*cos + perm(q)*sin_s, with the
    partition permutation done by 4 small SBUF->SBUF DMAs.
  - attention per (chunk, head-pair): S = q^T.T @ k^T in (query-part,
    key-free) layout for 4 window-groups at once (PSUM 128x512); exp on
    ACT; mask+rowsum in one scalar_tensor_tensor per group (accum_out);
    normalize; block-transpose P via the DVE 32x32 stream transpose
    (exact for the block-diagonal P); AV matmul with token-major v.
  - out proj + residual, DMA out.
"""

import numpy as np
import ml_dtypes

import concourse.bass as bass
import concourse.bacc as bacc
import concourse.tile as tile
import concourse.mybir as mybir
from concourse.bass_utils import run_bass_kernel_spmd

F32 = mybir.dt.float32
F32R = mybir.dt.float32r
BF16 = mybir.dt.bfloat16
NPBF16 = ml_dtypes.bfloat16
AF = mybir.ActivationFunctionType
ALU = mybir.AluOpType

DIM = 1024
T = 16384
NCORES = 8
TLOC = T // NCORES          # 2048
CHUNK = 512
NCHUNK = TLOC // CHUNK      # 4
HEADS = 16
DH = 64
WIN = 32
NPAIR = HEADS // 2          # 8 head pairs <-> 128-row tiles
NGRP = CHUNK // 128         # 4 groups of 128 tokens (4 windows each)
KT = DIM // 128             # 8 k-tiles of the contraction dim
EPS = 1e-5
SCALE = DH ** -0.5          # 0.125

_CACHE = {}


def _build(beta_nonzero: bool, opts: dict | None = None):
    O = dict(wo_stream=False, qe_bufs=1, vtok_bufs=1, ao_bufs=2,
             xb_bufs=3, normed_bufs=2, psmm_bufs=3, attn_inplace=False,
             p_bufs=3, rot_pool=False, sct_pool=True,
             rotadd_pool=False, norm_pool=False, bc_share=True,
             ln_early=False, s_bufs=2, rt1_pool=False,
             res_pool=True)
    if opts:
        O.update(opts)
    nc = bacc.Bacc("TRN2", target_bir_lowering=False, debug=False,
                   num_devices=NCORES)

    x_d = nc.dram_tensor("x", [DIM, TLOC], F32, kind="ExternalInput").ap()
    wq_d = nc.dram_tensor("wqkvT", [DIM, 3 * DIM], BF16,
                          kind="ExternalInput").ap()
    wo_d = nc.dram_tensor("woutT", [DIM, DIM], BF16, kind="ExternalInput").ap()
    cos_d = nc.dram_tensor("cosT", [128, CHUNK], BF16,
                           kind="ExternalInput").ap()
    sin_d = nc.dram_tensor("sinT", [128, CHUNK], BF16,
                           kind="ExternalInput").ap()
    msk_d = nc.dram_tensor("maskT", [128, 128], BF16,
                           kind="ExternalInput").ap()
    mskf_d = nc.dram_tensor("maskF", [128, CHUNK], BF16,
                            kind="ExternalInput").ap()
    ones_d = nc.dram_tensor("onesAB", [128, 33, 2], BF16,
                            kind="ExternalInput").ap()
    oner_d = nc.dram_tensor("onesrow", [1, 128], BF16,
                            kind="ExternalInput").ap()
    qb_d = nc.dram_tensor("qkvbias", [3 * DIM], F32, kind="ExternalInput").ap()
    vb_d = nc.dram_tensor("vbias", [128, DIM], BF16, kind="ExternalInput").ap()
    out_d = nc.dram_tensor("out", [DIM, TLOC], F32, kind="ExternalOutput").ap()

    x_v = x_d.rearrange("(t p) n -> p t n", p=128)       # (128, 8, 2048)
    wq_v = wq_d.rearrange("(t p) n -> p t n", p=128)     # (128, 8, 3072)
    wo_v = wo_d.rearrange("(t p) n -> p t n", p=128)     # (128, 8, 1024)
    qb_v = qb_d.rearrange("(t p) -> p t", p=128)         # (128, 24)
    out_v = out_d.rearrange("(t p) n -> p t n", p=128)   # (128, 8, 2048)

    from contextlib import ExitStack

    with tile.TileContext(nc) as tc:
        with ExitStack() as stk:
            ec = stk.enter_context
            wpool = ec(tc.tile_pool(name="weights", bufs=1))
            cpool = ec(tc.tile_pool(name="consts", bufs=1))
            xpool = ec(tc.tile_pool(name="xin", bufs=O["xb_bufs"]))
            xtpool = ec(tc.tile_pool(name="xt", bufs=2))
            xsqpool = ec(tc.tile_pool(name="xsq", bufs=1))
            lnrow = ec(tc.tile_pool(name="lnrow", bufs=1))
            lntmp = ec(tc.tile_pool(name="lntmp", bufs=1))
            npool = ec(tc.tile_pool(name="normed", bufs=O["normed_bufs"]))
            qepool = ec(tc.tile_pool(name="qkevict", bufs=2))
            qallpool = ec(tc.tile_pool(name="qall", bufs=O["qe_bufs"]))
            qppool = ec(tc.tile_pool(name="qperm", bufs=1))
            vpool = ec(tc.tile_pool(name="vtok", bufs=O["vtok_bufs"]))
            ppool = ec(tc.tile_pool(name="attnP", bufs=O["p_bufs"]))
            zpool = ec(tc.tile_pool(name="attnZ", bufs=2))
            aopool = ec(tc.tile_pool(name="ao", bufs=O["ao_bufs"]))
            opool = ec(tc.tile_pool(name="outs", bufs=2))
            ps_mm = ec(tc.tile_pool(name="ps_mm", bufs=O["psmm_bufs"],
                                    space="PSUM"))
            ps_stats = ec(tc.tile_pool(name="ps_stats", bufs=1, space="PSUM"))
            ps_bc = ec(tc.tile_pool(name="ps_bc", bufs=1, space="PSUM"))
            ps_s = ec(tc.tile_pool(name="ps_s", bufs=O["s_bufs"], space="PSUM"))
            ps_av = ec(tc.tile_pool(name="ps_av", bufs=1, space="PSUM"))

            # ---- constants (weights are loaded after chunk 0's x) ----
            cos_sb = cpool.tile([128, CHUNK], BF16, tag="cos")
            nc.sync.dma_start(cos_sb, cos_d)
            sin_sb = cpool.tile([128, CHUNK], BF16, tag="sin")
            nc.sync.dma_start(sin_sb, sin_d)
            msk_sb = cpool.tile([128, 128], BF16, tag="mask")
            nc.sync.dma_start(msk_sb, msk_d)
            mskf_sb = cpool.tile([128, CHUNK], BF16, tag="maskF")
            nc.sync.dma_start(mskf_sb, mskf_d)
            ones_sb = cpool.tile([128, 33, 2], BF16, tag="onesAB")
            nc.sync.dma_start(ones_sb, ones_d)
            oner_sb = cpool.tile([1, 128], BF16, tag="onesrow")
            nc.sync.dma_start(oner_sb, oner_d)
            qb_sb = cpool.tile([128, 24], F32, tag="qbias")
            nc.sync.dma_start(qb_sb, qb_v)
            vb_sb = None
            if beta_nonzero:
                vb_sb = cpool.tile([128, DIM], BF16, tag="vbias")
                nc.sync.dma_start(vb_sb, vb_d)
            eps_sb = cpool.tile([1, 1], F32, tag="eps")
            nc.vector.memset(eps_sb, EPS)
            wq_sb = wpool.tile([128, KT, 3 * DIM], BF16, tag="wq")
            wo_res = None
            if not O["wo_stream"]:
                wo_res = wpool.tile([128, KT, DIM], BF16, tag="wo")

            def load_weights():
                for t in range(KT):
                    nc.sync.dma_start(wq_sb[:, t, :], wq_v[:, t, :])
                if wo_res is not None:
                    nc.sync.dma_start(wo_res, wo_v)

            def ln_phase(ic):
                """LN stats + broadcasts for chunk ic -> (xb, a_sb, b2_sb)."""
                csl = slice(ic * CHUNK, (ic + 1) * CHUNK)
                xb = xpool.tile([128, KT, CHUNK], BF16, tag="xb")
                stats = ps_stats.tile([33, CHUNK], F32, tag="stats")
                for t in range(KT):
                    xt = xtpool.tile([128, CHUNK], F32, tag="xt")
                    nc.sync.dma_start(xt, x_v[:, t, csl])
                    nc.scalar.copy(xb[:, t, :], xt)
                    nc.tensor.matmul(stats, ones_sb[:, :, 0], xb[:, t, :],
                                     start=(t == 0), stop=False)
                for t in range(KT):
                    xsq = xsqpool.tile([128, CHUNK], BF16, tag="xsq")
                    nc.vector.tensor_mul(xsq, xb[:, t, :], xb[:, t, :])
                    nc.tensor.matmul(stats, ones_sb[:, :, 1], xsq,
                                     start=False, stop=(t == KT - 1))

                mu = lnrow.tile([1, CHUNK], F32, tag="mu")
                nc.vector.tensor_scalar_mul(mu, stats[0:1, :], 1.0 / DIM)
                var = lnrow.tile([1, CHUNK], F32, tag="var")
                nc.vector.tensor_mul(var, mu, mu)
                nc.vector.scalar_tensor_tensor(var, stats[32:33, :],
                                               1.0 / DIM, var,
                                               ALU.mult, ALU.subtract)
                nc.scalar.activation(var, var, AF.Sqrt, bias=eps_sb)
                a_row = lnrow.tile([1, CHUNK], F32, tag="arow")
                nc.vector.reciprocal(a_row, var)
                b2_row = lnrow.tile([1, CHUNK], F32, tag="b2row")
                nc.vector.scalar_tensor_tensor(b2_row, mu, -1.0, a_row,
                                               ALU.mult, ALU.mult)

                def bcast(row, tag):
                    hi = lnrow.tile([1, CHUNK], BF16, tag=tag + "hi")
                    nc.vector.tensor_copy(hi, row)
                    bc = ps_bc.tile([128, CHUNK], F32,
                                    tag=("bc" if O["bc_share"] else tag))
                    nc.tensor.matmul(bc, oner_sb, hi, start=True, stop=True)
                    sb = lntmp.tile([128, CHUNK], BF16, tag=tag + "sb",
                                    bufs=1)
                    nc.scalar.copy(sb, bc)
                    return sb

                a_sb = bcast(a_row, "abc")
                b2_sb = bcast(b2_row, "b2bc")
                return xb, a_sb, b2_sb

            def ln_apply_tile(st, t):
                xb, a_sb, b2_sb = st["ln"]
                tmp = lntmp.tile([128, CHUNK], BF16, tag="lntmp")
                nc.vector.tensor_mul(tmp, xb[:, t, :], a_sb)
                nc.vector.tensor_add(st["normed"][:, t, :], tmp, b2_sb)

            def ln_apply(st):
                for t in range(KT):
                    ln_apply_tile(st, t)

            def qk_tile(st, jp):
                """project q/k tile jp of chunk st and evict raw to qe_all."""
                normed = st["normed"]
                ps = ps_mm.tile([128, CHUNK], F32, tag="mm")
                for t in range(KT):
                    nc.tensor.matmul(
                        ps, wq_sb[:, t, jp * 128:(jp + 1) * 128],
                        normed[:, t, :], start=(t == 0), stop=(t == KT - 1))
                nc.scalar.activation(st["qe"][:, jp, :], ps, AF.Identity,
                                     bias=qb_sb[:, jp:jp + 1])

            def perm_quarter(st, hs):
                for a in range(4):
                    src = (a // 2) * 64 + ((a % 2) ^ 1) * 32
                    nc.sync.dma_start(
                        st["qp"][a * 32:(a + 1) * 32, hs, :],
                        st["qe"][src:src + 32, hs, :])

            def rotary(st, jp):
                t1 = qepool.tile([128, CHUNK], BF16, tag="rt1")
                (nc.gpsimd if O["rt1_pool"] else nc.vector).tensor_mul(
                    t1, st["qe"][:, jp, :], cos_sb)
                t2 = qepool.tile([128, CHUNK], BF16, tag="rt2")
                if O["rot_pool"]:
                    nc.gpsimd.tensor_mul(t2, st["qp"][:, jp, :], sin_sb)
                    nc.gpsimd.tensor_add(st["qe"][:, jp, :], t1, t2)
                else:
                    nc.vector.tensor_mul(t2, st["qp"][:, jp, :], sin_sb)
                    (nc.gpsimd if O["rotadd_pool"] else nc.vector).tensor_add(
                        st["qe"][:, jp, :], t1, t2)

            def v_tile(st, g):
                normed = st["normed"]
                for hf in range(2):
                    ps = ps_mm.tile([128, CHUNK], F32, tag="mm")
                    for t in range(KT):
                        nc.tensor.matmul(
                            ps, normed[:, t, g * 128:(g + 1) * 128],
                            wq_sb[:, t, 2 * DIM + hf * CHUNK:
                                  2 * DIM + (hf + 1) * CHUNK],
                            start=(t == 0), stop=(t == KT - 1))
                    vdst = st["vt"][:, g, hf * CHUNK:(hf + 1) * CHUNK]
                    nc.scalar.copy(vdst, ps)
                    if beta_nonzero:
                        nc.vector.scalar_tensor_tensor(
                            vdst, vb_sb[:, hf * CHUNK:(hf + 1) * CHUNK],
                            1.0, vdst, ALU.mult, ALU.add)

            def attn_s(st, p):
                """S matmuls for head pair p (both heads)."""
                s_ab = []
                for h2 in range(2):
                    s_ps = ps_s.tile([128, CHUNK], F32, tag="s")
                    rs = slice(h2 * 64, (h2 + 1) * 64)
                    for g in range(NGRP):
                        gs = slice(g * 128, (g + 1) * 128)
                        nc.tensor.matmul(
                            s_ps[:, gs], st["qe"][rs, p, gs],
                            st["qe"][rs, NPAIR + p, gs],
                            start=True, stop=True)
                    s_ab.append(s_ps)
                st["s"][p] = s_ab

            def attn_soft(st, p):
                """softmax chain for pair p -> PT tiles."""
                pts = []
                for h2 in range(2):
                    pe_ = ppool.tile([128, CHUNK], BF16, tag="pexp")
                    nc.scalar.activation(pe_, st["s"][p][h2], AF.Exp,
                                         scale=SCALE)
                    z = zpool.tile([128, NGRP], F32, tag="z")
                    pm = (pe_ if O["attn_inplace"] else
                          ppool.tile([128, CHUNK], BF16, tag="pm"))
                    if O["sct_pool"]:
                        # mask on Pool (plain tensor_tensor), Z via DVE
                        # reduce -- walrus rejects TensorScalarPtr on Pool.
                        nc.gpsimd.tensor_tensor(pm, pe_, mskf_sb, ALU.mult)
                        nc.vector.tensor_reduce(
                            z, pm.rearrange("p (g n) -> p g n", g=NGRP),
                            axis=mybir.AxisListType.X, op=ALU.add)
                    else:
                        for g in range(NGRP):
                            gs = slice(g * 128, (g + 1) * 128)
                            nc.vector.scalar_tensor_tensor(
                                pm[:, gs], pe_[:, gs], 1.0, msk_sb,
                                ALU.mult, ALU.mult, accum_out=z[:, g:g + 1])
                    rz = zpool.tile([128, NGRP], F32, tag="rz")
                    nc.vector.reciprocal(rz, z)
                    pmv = pm.rearrange("p (g n) -> p g n", g=NGRP)
                    pn = (pm if O["attn_inplace"] else
                          ppool.tile([128, NGRP, 128], BF16, tag="pn"))
                    pnv = pn.rearrange("p (g n) -> p g n", g=NGRP) \
                        if O["attn_inplace"] else pn
                    (nc.gpsimd if O["norm_pool"] else nc.vector).tensor_tensor(
                        pnv, pmv,
                        rz[:, :, None].to_broadcast((128, NGRP, 128)),
                        ALU.mult)
                    pt = ppool.tile([128, CHUNK], BF16, tag="pt", bufs=4)
                    nc.vector.transpose(
                        pt, pn if O["attn_inplace"]
                        else pn.rearrange("p g n -> p (g n)"))
                    pts.append(pt)
                st["pt"][p] = pts
                st["s"][p] = None

            def attn_av(st, p):
                av = ps_av.tile([128, CHUNK], F32, tag="av")
                for h2 in range(2):
                    cv = slice((2 * p + h2) * DH, (2 * p + h2 + 1) * DH)
                    for g in range(NGRP):
                        gs = slice(g * 128, (g + 1) * 128)
                        nc.tensor.matmul(
                            av[h2 * 64:(h2 + 1) * 64, gs],
                            st["vt"][:, g, cv], st["pt"][p][h2][:, gs],
                            start=True, stop=True,
                            tile_position=(0, h2 * 64))
                nc.scalar.copy(st["ao"][:, p, :], av)
                st["pt"][p] = None

            def proj_tile(st, j):
                ic = st["ic"]
                csl = slice(ic * CHUNK, (ic + 1) * CHUNK)
                if O["wo_stream"]:
                    wo_sb = qepool.tile([128, KT, 128], BF16, tag="wo",
                                        bufs=2)
                    nc.sync.dma_start(wo_sb,
                                      wo_v[:, :, j * 128:(j + 1) * 128])
                ps = ps_mm.tile([128, CHUNK], F32, tag="mm")
                for t in range(KT):
                    nc.tensor.matmul(
                        ps,
                        wo_sb[:, t, :] if O["wo_stream"]
                        else wo_res[:, t, j * 128:(j + 1) * 128],
                        st["ao"][:, t, :], start=(t == 0), stop=(t == KT - 1))
                xr = xtpool.tile([128, CHUNK], F32, tag="xr", bufs=2)
                nc.sync.dma_start(xr, x_v[:, j, csl])
                o = opool.tile([128, CHUNK], F32, tag="o")
                if O["res_pool"]:
                    nc.scalar.copy(o, ps)
                    nc.gpsimd.tensor_add(o, o, xr)
                else:
                    nc.vector.tensor_tensor(o, ps, xr, ALU.add)
                nc.sync.dma_start(out_v[:, j, csl], o)

            def new_state(ic):
                return {
                    "ic": ic,
                    "ln": ln_phase(ic),
                    "normed": npool.tile([128, KT, CHUNK], BF16,
                                         tag="normed", name=f"normed{ic}"),
                    "qe": qallpool.tile([128, 2 * NPAIR, CHUNK], BF16,
                                        tag="qeall", name=f"qeall{ic}"),
                    "qp": qppool.tile([128, 2 * NPAIR, CHUNK], BF16,
                                      tag="qpall", name=f"qpall{ic}"),
                    "vt": vpool.tile([128, NGRP, DIM], BF16, tag="vtok",
                                     name=f"vtok{ic}"),
                    "ao": aopool.tile([128, NPAIR, CHUNK], BF16, tag="ao",
                                      name=f"ao{ic}"),
                    "s": [None] * NPAIR,
                    "pt": [None] * NPAIR,
                }

            # ---- software pipeline over chunks ----
            # super-iteration i: QKV/rotary/v for chunk i interleaved with
            # attention for chunk i-1, then proj for chunk i-1.  The LN
            # stats/apply for chunk i+1 are prefetched near the end of
            # super-iteration i so the next iteration's QKV can start
            # immediately.
            prev = None
            cur = new_state(0)
            ln_apply(cur)
            load_weights()
            for ic in range(NCHUNK):
                # QKV for cur; rotary per quarter as soon as its perm is in
                for p in range(NPAIR):
                    qk_tile(cur, p)           # q tile p
                    qk_tile(cur, NPAIR + p)   # k tile p
                    if p % 4 == 3:
                        q0 = p - 3
                        perm_quarter(cur, slice(q0, q0 + 4))
                        perm_quarter(cur, slice(NPAIR + q0, NPAIR + q0 + 4))
                        for pp in range(q0, q0 + 4):
                            rotary(cur, pp)
                            rotary(cur, NPAIR + pp)
                # V for cur interleaved with attention S/softmax for cur
                for g in range(NGRP):
                    v_tile(cur, g)
                    attn_s(cur, 2 * g)
                    attn_soft(cur, 2 * g)
                    attn_s(cur, 2 * g + 1)
                    attn_soft(cur, 2 * g + 1)
                # AV for cur interleaved with proj for prev
                for p in range(NPAIR):
                    attn_av(cur, p)
                    if prev is not None:
                        proj_tile(prev, p)
                nxt = None
                if ic + 1 < NCHUNK:
                    nxt = new_state(ic + 1)
                    ln_apply(nxt)
                prev = cur
                cur = nxt

            # drain: proj for the last chunk
            for j in range(KT):
                proj_tile(prev, j)

    nc.compile()
    return nc


def _host_constants(w_qkv, w_out, gamma, beta):
    wg = (w_qkv.astype(np.float32) * gamma.astype(np.float32)[None, :])
    wqkvT = np.ascontiguousarray(wg.T).astype(NPBF16)            # (1024,3072)
    woutT = np.ascontiguousarray(w_out.astype(np.float32).T).astype(NPBF16)
    qkvbias = (w_qkv.astype(np.float32) @ beta.astype(np.float32)
               ).astype(np.float32)                              # (3072,)
    vbias = np.ascontiguousarray(
        np.broadcast_to(qkvbias[2 * DIM:].astype(NPBF16), (128, DIM)))

    inv_freq = (1.0 / (10000.0 ** (np.arange(0, DH, 2, dtype=np.float64)
                                   / DH))).astype(np.float64)    # (32,)
    p = np.arange(128)
    j = np.arange(CHUNK)
    pos = (j % WIN).astype(np.float64)
    freq = inv_freq[(p % DH) % 32]                               # (128,)
    ang = freq[:, None] * pos[None, :]                           # (128, 512)
    cosT = np.cos(ang).astype(NPBF16)
    sgn = np.where((p % DH) < 32, -1.0, 1.0)
    sinT = (sgn[:, None] * np.sin(ang)).astype(NPBF16)

    mask = ((p[:, None] // WIN) == (np.arange(128)[None, :] // WIN)
            ).astype(NPBF16)                                     # (128,128)
    maskF = np.ascontiguousarray(np.tile(mask, (1, CHUNK // 128)))

    onesAB = np.zeros((128, 33, 2), NPBF16)
    onesAB[:, 0, 0] = 1.0
    onesAB[:, 32, 1] = 1.0
    onesrow = np.ones((1, 128), NPBF16)
    return dict(wqkvT=wqkvT, woutT=woutT, qkvbias=qkvbias, vbias=vbias,
                cosT=cosT, sinT=sinT, maskT=mask, maskF=maskF,
                onesAB=onesAB, onesrow=onesrow)


def _run(inputs, trace=False, trace_cores=None):
    x = np.asarray(inputs["x"], dtype=np.float32)
    consts = _host_constants(np.asarray(inputs["w_qkv"], np.float32),
                             np.asarray(inputs["w_out"], np.float32),
                             np.asarray(inputs["gamma"], np.float32),
                             np.asarray(inputs["beta"], np.float32))
    beta_nonzero = bool(np.any(np.asarray(inputs["beta"]) != 0))
    key = ("nc", beta_nonzero)
    if key not in _CACHE:
        _CACHE[key] = _build(beta_nonzero)
    nc = _CACHE[key]

    in_maps = []
    for c in range(NCORES):
        m = dict(consts)
        m["x"] = np.ascontiguousarray(x[:, c * TLOC:(c + 1) * TLOC])
        if not beta_nonzero:
            m.pop("vbias")
            m["vbias"] = np.zeros((128, DIM), NPBF16)
        in_maps.append(m)

    res = run_bass_kernel_spmd(nc, in_maps, list(range(NCORES)),
                               trace=trace,
                               trace_cores=trace_cores)
    out = np.concatenate([res.results[c]["out"] for c in range(NCORES)],
                         axis=1)
    return out, res


def kernel(**inputs):
    out, _ = _run(inputs)
    return out



# revision 23
# speedup vs baseline: 1.2042x; 1.2042x over previous
"""Trainium2 Bass kernel for local windowed MHA (nn_LocalMHA).

Computation (see reference): x (C=1024, T=16384) -> LayerNorm over C ->
QKV proj -> rotary (window-relative) -> per-head attention within windows
of 32 tokens -> out proj -> +x residual.

Sharding: T split across 8 cores (2048 tokens each); windows are local so
no communication is needed. Weights replicated.

v2 design (per core, per 512-token chunk):
  - x DMA-cast to bf16 (gpsimd dma); LN stats via ones-matmul; short row
    chain (Act Rsqrt) emitting bf16 rows; per-token a/b2 broadcast via K=1
    matmuls; ln_apply 2 DVE ops -> normed bf16; n8/nr8 fp8 pair (x8 + res)
    for error-compensated fp8 matmuls.
  - QKV q/k + v: fp8 e4m3 DoubleRow matmuls (K=256 per mm), 3-term
    compensation (x8*W8 + x8*Wr8 + r8*W8) accumulated in one PSUM group;
    eviction descales on ACT.  Out-proj: plain fp8 DoubleRow.
  - rotary on eviction: q_rot = q*cos + perm(q)*sin; muls on DVE, add on
    Pool; partition permutation via 4 SBUF->SBUF DMAs.
  - attention computed TRANSPOSED (S^T = k^T q per 128-token group):
    window mask added as a rank-5 K=5 matmul into the same PSUM; exp on
    ACT gives P^T directly in AV-ready layout (no stream transpose);
    Z via ones-matmul broadcast to both 64-row halves; one DVE reciprocal;
    AV matmul; normalization folded into the fp8 ao eviction (DVE stt).
  - out proj + residual (Pool add), DMA out.
"""

import numpy as np
import ml_dtypes

import concourse.bass as bass
import concourse.bacc as bacc
import concourse.tile as tile
import concourse.mybir as mybir
from concourse.bass_utils import run_bass_kernel_spmd

F32 = mybir.dt.float32
BF16 = mybir.dt.bfloat16
F8 = mybir.dt.float8e4
NPBF16 = ml_dtypes.bfloat16
NPF8 = ml_dtypes.float8_e4m3
AF = mybir.ActivationFunctionType
ALU = mybir.AluOpType
DR = mybir.MatmulPerfMode.DoubleRow

DIM = 1024
T = 16384
NCORES = 8
TLOC = T // NCORES          # 2048
CHUNK = 512
NCHUNK = TLOC // CHUNK      # 4
HEADS = 16
DH = 64
WIN = 32
NPAIR = HEADS // 2          # 8 head pairs <-> 128-row tiles
NGRP = CHUNK // 128         # 4 groups of 128 tokens (4 windows each)
KT = DIM // 128             # 8 k-tiles of the contraction dim
KP = KT // 2                # 4 DoubleRow k-pairs
EPS = 1e-5
SCALE = DH ** -0.5          # 0.125
SX = 16.0                   # fp8 scale for activations
SW = 1024.0                 # fp8 scale for weights
MBIG = 800.0                # additive mask in raw-logit units (=100 post
                            # scale); exp(-100+..) flushes to 0 in bf16

_CACHE = {}


def _build(beta_nonzero: bool, opts: dict | None = None):
    O = dict(xb_bufs=3, n8_bufs=2, normed_bufs=2, psmm_bufs=2, s_bufs=2,
             p_bufs=4, ao_bufs=2, rotadd_pool=True, res_pool=True,
             lnadd_pool=True, ao8_pool=False, xb_cast=True,
             qk_mode="comp", v_mode="comp", proj_mode="fp8")
    if opts:
        O.update(opts)
    nc = bacc.Bacc("TRN2", target_bir_lowering=False, debug=False,
                   num_devices=NCORES)

    x_d = nc.dram_tensor("x", [DIM, TLOC], F32, kind="ExternalInput").ap()
    # fp8 weights: hi and residual ("comp" terms).  qk: 2048 out-ch,
    # v: 1024 out-ch, wo: 1024 out-ch.
    wqk8_d = nc.dram_tensor("wqk8", [DIM, 2 * DIM], F8,
                            kind="ExternalInput").ap()
    wqkr_d = nc.dram_tensor("wqkr", [DIM, 2 * DIM], F8,
                            kind="ExternalInput").ap()
    wv8_d = nc.dram_tensor("wv8", [DIM, DIM], F8, kind="ExternalInput").ap()
    wvr_d = nc.dram_tensor("wvr", [DIM, DIM], F8, kind="ExternalInput").ap()
    wo8_d = nc.dram_tensor("wo8", [DIM, DIM], F8, kind="ExternalInput").ap()
    wor_d = nc.dram_tensor("wor", [DIM, DIM], F8, kind="ExternalInput").ap()
    cos_d = nc.dram_tensor("cosT", [128, CHUNK], BF16,
                           kind="ExternalInput").ap()
    sin_d = nc.dram_tensor("sinT", [128, CHUNK], BF16,
                           kind="ExternalInput").ap()
    mskL_d = nc.dram_tensor("maskL", [5, 128], BF16,
                            kind="ExternalInput").ap()
    mskR_d = nc.dram_tensor("maskR", [5, CHUNK], BF16,
                            kind="ExternalInput").ap()
    ones_d = nc.dram_tensor("onesAB", [128, 33, 2], BF16,
                            kind="ExternalInput").ap()
    on64_d = nc.dram_tensor("ones64", [128, DH], BF16,
                            kind="ExternalInput").ap()
    oner_d = nc.dram_tensor("onesrow", [1, 128], BF16,
                            kind="ExternalInput").ap()
    idsc_d = nc.dram_tensor("idscaled", [128, 128], BF16,
                            kind="ExternalInput").ap()
    qb_d = nc.dram_tensor("qkvbias", [3 * DIM], F32, kind="ExternalInput").ap()
    vb_d = nc.dram_tensor("vbias", [128, DIM], BF16, kind="ExternalInput").ap()
    out_d = nc.dram_tensor("out", [DIM, TLOC], F32, kind="ExternalOutput").ap()
    dbg = O0 = (opts or {}).get("debug_dump", False)
    if dbg:
        dbg_qe = nc.dram_tensor("dbg_qe", [128, 16, CHUNK], BF16,
                                kind="ExternalOutput").ap()
        dbg_vt = nc.dram_tensor("dbg_vt", [128, NGRP, DIM], BF16,
                                kind="ExternalOutput").ap()
        dbg_pt = nc.dram_tensor("dbg_pt", [128, 2, CHUNK], BF16,
                                kind="ExternalOutput").ap()
        dbg_rz = nc.dram_tensor("dbg_rz", [128, CHUNK], BF16,
                                kind="ExternalOutput").ap()
        dbg_ao = nc.dram_tensor("dbg_ao", [128, NPAIR, CHUNK], F8,
                                kind="ExternalOutput").ap()
        dbg_n8 = nc.dram_tensor("dbg_n8", [128, KT, CHUNK], F8,
                                kind="ExternalOutput").ap()
        dbg_nr8 = nc.dram_tensor("dbg_nr8", [128, KT, CHUNK], F8,
                                 kind="ExternalOutput").ap()
        dbg_ab = nc.dram_tensor("dbg_ab", [128, 2, CHUNK], BF16,
                                kind="ExternalOutput").ap()

    x_v = x_d.rearrange("(t p) n -> p t n", p=128)        # (128, 8, 2048)
    wqk8_v = wqk8_d.rearrange("(t p) n -> p t n", p=128)  # (128, 8, 2048)
    wqkr_v = wqkr_d.rearrange("(t p) n -> p t n", p=128)
    wv8_v = wv8_d.rearrange("(t p) n -> p t n", p=128)    # (128, 8, 1024)
    wvr_v = wvr_d.rearrange("(t p) n -> p t n", p=128)
    wo8_v = wo8_d.rearrange("(t p) n -> p t n", p=128)
    wor_v = wor_d.rearrange("(t p) n -> p t n", p=128)
    qb_v = qb_d.rearrange("(t p) -> p t", p=128)          # (128, 24)
    out_v = out_d.rearrange("(t p) n -> p t n", p=128)    # (128, 8, 2048)

    QK_COMP = O["qk_mode"] == "comp"
    V_COMP = O["v_mode"] == "comp"
    PJ_COMP = O["proj_mode"] == "comp"
    NEED_NR = QK_COMP or V_COMP

    from contextlib import ExitStack

    with tile.TileContext(nc) as tc:
        with ExitStack() as stk:
            ec = stk.enter_context
            wpool = ec(tc.tile_pool(name="weights", bufs=1))
            cpool = ec(tc.tile_pool(name="consts", bufs=1))
            xpool = ec(tc.tile_pool(name="xin", bufs=O["xb_bufs"]))
            xsqpool = ec(tc.tile_pool(name="xsq", bufs=1))
            lnrow = ec(tc.tile_pool(name="lnrow", bufs=1))
            lntmp = ec(tc.tile_pool(name="lntmp", bufs=1))
            npool = ec(tc.tile_pool(name="normed", bufs=O["normed_bufs"]))
            n8pool = ec(tc.tile_pool(name="n8", bufs=O["n8_bufs"]))
            qallpool = ec(tc.tile_pool(name="qall", bufs=1))
            qppool = ec(tc.tile_pool(name="qperm", bufs=1))
            qepool = ec(tc.tile_pool(name="qkevict", bufs=2))
            vpool = ec(tc.tile_pool(name="vtok", bufs=1))
            ppool = ec(tc.tile_pool(name="attnP", bufs=O["p_bufs"]))
            zpool = ec(tc.tile_pool(name="attnZ", bufs=2))
            aopool = ec(tc.tile_pool(name="ao", bufs=O["ao_bufs"]))
            opool = ec(tc.tile_pool(name="outs", bufs=2))
            xtpool = ec(tc.tile_pool(name="xt", bufs=2))
            ps_mm = ec(tc.tile_pool(name="ps_mm", bufs=O["psmm_bufs"],
                                    space="PSUM"))
            ps_stats = ec(tc.tile_pool(name="ps_stats", bufs=1, space="PSUM"))
            ps_bc = ec(tc.tile_pool(name="ps_bc", bufs=1, space="PSUM"))
            ps_s = ec(tc.tile_pool(name="ps_s", bufs=O["s_bufs"],
                                   space="PSUM"))
            ps_z = ec(tc.tile_pool(name="ps_z", bufs=1, space="PSUM"))
            ps_av = ec(tc.tile_pool(name="ps_av", bufs=1, space="PSUM"))

            # ---- constants ----
            cos_sb = cpool.tile([128, CHUNK], BF16, tag="cos")
            nc.sync.dma_start(cos_sb, cos_d)
            sin_sb = cpool.tile([128, CHUNK], BF16, tag="sin")
            nc.sync.dma_start(sin_sb, sin_d)
            mskL_sb = cpool.tile([5, 128], BF16, tag="mskL")
            nc.sync.dma_start(mskL_sb, mskL_d)
            mskR_sb = cpool.tile([5, CHUNK], BF16, tag="mskR")
            nc.sync.dma_start(mskR_sb, mskR_d)
            ones_sb = cpool.tile([128, 33, 2], BF16, tag="onesAB")
            nc.sync.dma_start(ones_sb, ones_d)
            on64_sb = cpool.tile([128, DH], BF16, tag="ones64")
            nc.sync.dma_start(on64_sb, on64_d)
            oner_sb = cpool.tile([1, 128], BF16, tag="onesrow")
            nc.sync.dma_start(oner_sb, oner_d)
            idsc_sb = cpool.tile([128, 128], BF16, tag="idscaled")
            nc.sync.dma_start(idsc_sb, idsc_d)
            qb_sb = cpool.tile([128, 24], F32, tag="qbias")
            nc.sync.dma_start(qb_sb, qb_v)
            vb_sb = None
            if beta_nonzero:
                vb_sb = cpool.tile([128, DIM], BF16, tag="vbias")
                nc.sync.dma_start(vb_sb, vb_d)
            eps_sb = cpool.tile([1, 1], F32, tag="eps")
            nc.vector.memset(eps_sb, EPS)

            wqk8_sb = wpool.tile([128, KT, 2 * DIM], F8, tag="wqk8")
            wqkr_sb = wpool.tile([128, KT, 2 * DIM], F8, tag="wqkr",
                                 name="wqkr_sb") if QK_COMP else None
            wv8_sb = wpool.tile([128, KT, DIM], F8, tag="wv8")
            wvr_sb = wpool.tile([128, KT, DIM], F8, tag="wvr",
                                name="wvr_sb") if V_COMP else None
            wo8_sb = wpool.tile([128, KT, DIM], F8, tag="wo8")
            wor_sb = wpool.tile([128, KT, DIM], F8, tag="wor",
                                name="wor_sb") if PJ_COMP else None

            def load_weights():
                # per-column-block so early qk tiles can start asap
                for jp in range(2 * NPAIR):
                    cs = slice(jp * 128, (jp + 1) * 128)
                    nc.sync.dma_start(wqk8_sb[:, :, cs], wqk8_v[:, :, cs])
                    if QK_COMP:
                        nc.sync.dma_start(wqkr_sb[:, :, cs], wqkr_v[:, :, cs])
                nc.sync.dma_start(wv8_sb, wv8_v)
                if V_COMP:
                    nc.sync.dma_start(wvr_sb, wvr_v)
                nc.sync.dma_start(wo8_sb, wo8_v)
                if PJ_COMP:
                    nc.sync.dma_start(wor_sb, wor_v)

            def ln_phase(ic):
                """LN stats for chunk ic -> (xb, a_sb, b2_sb)."""
                csl = slice(ic * CHUNK, (ic + 1) * CHUNK)
                xb = xpool.tile([128, KT, CHUNK], BF16, tag="xb")
                stats = ps_stats.tile([33, CHUNK], F32, tag="stats")
                for t in range(KT):
                    if O["xb_cast"]:
                        nc.gpsimd.dma_start(xb[:, t, :], x_v[:, t, csl])
                    else:
                        xt = xtpool.tile([128, CHUNK], F32, tag="xt")
                        nc.sync.dma_start(xt, x_v[:, t, csl])
                        nc.scalar.copy(xb[:, t, :], xt)
                    nc.tensor.matmul(stats, ones_sb[:, :, 0], xb[:, t, :],
                                     start=(t == 0), stop=False)
                for t in range(KT):
                    xsq = xsqpool.tile([128, CHUNK], BF16, tag="xsq")
                    nc.vector.tensor_mul(xsq, xb[:, t, :], xb[:, t, :])
                    nc.tensor.matmul(stats, ones_sb[:, :, 1], xsq,
                                     start=False, stop=(t == KT - 1))

                mu = lnrow.tile([1, CHUNK], F32, tag="mu")
                nc.vector.tensor_scalar_mul(mu, stats[0:1, :], 1.0 / DIM)
                var = lnrow.tile([1, CHUNK], F32, tag="var")
                nc.vector.tensor_mul(var, mu, mu)
                nc.vector.scalar_tensor_tensor(var, stats[32:33, :],
                                               1.0 / DIM, var,
                                               ALU.mult, ALU.subtract)
                nc.scalar.activation(var, var, AF.Sqrt, bias=eps_sb)
                a_hi = lnrow.tile([1, CHUNK], BF16, tag="ahi")
                with nc.allow_low_precision(reason="rstd fits bf16"):
                    nc.vector.reciprocal(a_hi, var)
                b2_hi = lnrow.tile([1, CHUNK], BF16, tag="b2hi")
                nc.vector.scalar_tensor_tensor(b2_hi, mu, -1.0, a_hi,
                                               ALU.mult, ALU.mult)

                def bcast(hi, tag):
                    bc = ps_bc.tile([128, CHUNK], F32, tag="bc")
                    nc.tensor.matmul(bc, oner_sb, hi, start=True, stop=True)
                    sb = lntmp.tile([128, CHUNK], BF16, tag=tag + "sb",
                                    bufs=1)
                    nc.scalar.copy(sb, bc)
                    return sb

                a_sb = bcast(a_hi, "abc")
                b2_sb = bcast(b2_hi, "b2bc")
                return xb, a_sb, b2_sb

            def ln_apply_tile(st, t):
                """normalize x tile t, emit fp8 hi (+residual).

                a_sb/b2_sb carry a folded x16 so nrm16 = 16*normed; n8 is
                then a plain fp8 cast and nr8 a plain subtract.
                """
                xb, a_sb, b2_sb = st["ln"]
                tmp = lntmp.tile([128, CHUNK], BF16, tag="lntmp")
                nc.vector.tensor_mul(tmp, xb[:, t, :], a_sb)
                nrm = npool.tile([128, CHUNK], BF16, tag="nrm")
                (nc.gpsimd if O["lnadd_pool"] else nc.vector).tensor_add(
                    nrm, tmp, b2_sb)
                nc.scalar.copy(st["n8"][:, t, :], nrm)
                if NEED_NR:
                    nc.vector.tensor_tensor(
                        st["nr8"][:, t, :], nrm, st["n8"][:, t, :],
                        ALU.subtract)

            def ln_apply(st):
                for t in range(KT):
                    ln_apply_tile(st, t)

            def qk_tile(st, jp):
                """project q/k tile jp of chunk st, evict raw to qe."""
                n8, nr8 = st["n8"], st["nr8"]
                ps = ps_mm.tile([128, CHUNK], F32, tag="mm")
                cs = slice(jp * 128, (jp + 1) * 128)
                first = True
                terms = [(wqk8_sb, n8)]
                if QK_COMP:
                    terms += [(wqkr_sb, n8), (wqk8_sb, nr8)]
                nterm = len(terms)
                for ti, (w, a) in enumerate(terms):
                    for tp in range(KP):
                        ks = slice(2 * tp, 2 * tp + 2)
                        nc.tensor.matmul(
                            ps, w[:, ks, cs], a[:, ks, :],
                            start=first,
                            stop=(ti == nterm - 1 and tp == KP - 1),
                            perf_mode=DR)
                        first = False
                nc.scalar.activation(st["qe"][:, jp, :], ps, AF.Identity,
                                     scale=1.0 / (SX * SW),
                                     bias=qb_sb[:, jp:jp + 1])

            def perm_quarter(st, hs):
                for a in range(4):
                    src = (a // 2) * 64 + ((a % 2) ^ 1) * 32
                    nc.sync.dma_start(
                        st["qp"][a * 32:(a + 1) * 32, hs, :],
                        st["qe"][src:src + 32, hs, :])

            def rotary(st, jp):
                t1 = qepool.tile([128, CHUNK], BF16, tag="rt1")
                nc.vector.tensor_mul(t1, st["qe"][:, jp, :], cos_sb)
                t2 = qepool.tile([128, CHUNK], BF16, tag="rt2")
                nc.vector.tensor_mul(t2, st["qp"][:, jp, :], sin_sb)
                ra = O["rotadd_pool"]
                if ra == "split":
                    eng = nc.gpsimd if jp % 2 else nc.vector
                else:
                    eng = nc.gpsimd if ra else nc.vector
                eng.tensor_add(st["qe"][:, jp, :], t1, t2)

            def v_tile(st, g):
                n8, nr8 = st["n8"], st["nr8"]
                gs = slice(g * 128, (g + 1) * 128)
                for hf in range(2):
                    ps = ps_mm.tile([128, CHUNK], F32, tag="mm")
                    ws = slice(hf * CHUNK, (hf + 1) * CHUNK)
                    first = True
                    terms = [(n8, wv8_sb)]
                    if V_COMP:
                        terms += [(n8, wvr_sb), (nr8, wv8_sb)]
                    nterm = len(terms)
                    for ti, (a, w) in enumerate(terms):
                        for tp in range(KP):
                            ks = slice(2 * tp, 2 * tp + 2)
                            nc.tensor.matmul(
                                ps, a[:, ks, gs], w[:, ks, ws],
                                start=first,
                                stop=(ti == nterm - 1 and tp == KP - 1),
                                perf_mode=DR)
                            first = False
                    vdst = st["vt"][:, g, ws]
                    nc.scalar.activation(vdst, ps, AF.Copy,
                                         scale=1.0 / (SX * SW))
                    if beta_nonzero:
                        nc.vector.scalar_tensor_tensor(
                            vdst, vb_sb[:, ws], 1.0, vdst,
                            ALU.mult, ALU.add)

            def attn_pair(st, p):
                """S^T + exp + Z for pair p -> (pt tiles, rz)."""
                z_ps = ps_z.tile([128, CHUNK], F32, tag="z")
                pts = []
                for h2 in range(2):
                    rs = slice(h2 * 64, (h2 + 1) * 64)
                    s_ps = ps_s.tile([128, CHUNK], F32, tag="s")
                    # mask first: start=True zeroes the whole 2KB psum
                    # zero-region, so the group matmuls must accumulate
                    nc.tensor.matmul(s_ps, mskL_sb, mskR_sb,
                                     start=True, stop=False)
                    for g in range(NGRP):
                        gs = slice(g * 128, (g + 1) * 128)
                        nc.tensor.matmul(
                            s_ps[:, gs], st["qe"][rs, NPAIR + p, gs],
                            st["qe"][rs, p, gs],
                            start=False, stop=(g == NGRP - 1))
                    pt = ppool.tile([128, CHUNK], BF16, tag="pt")
                    nc.scalar.activation(pt, s_ps, AF.Exp, scale=SCALE)
                    nc.tensor.matmul(z_ps[rs, :], on64_sb, pt,
                                     start=True, stop=True,
                                     tile_position=(0, h2 * 64))
                    pts.append(pt)
                rz = zpool.tile([128, CHUNK], BF16, tag="rz")
                with nc.allow_low_precision(reason="1/Z fits bf16"):
                    nc.vector.reciprocal(rz, z_ps)
                if dbg and st["ic"] == 0 and p == 0:
                    for h2 in range(2):
                        nc.sync.dma_start(dbg_pt[:, h2, :], pts[h2])
                    nc.sync.dma_start(dbg_rz, rz)
                st["pt"][p] = pts
                st["rz"][p] = rz

            def attn_av(st, p):
                av = ps_av.tile([128, CHUNK], F32, tag="av")
                for h2 in range(2):
                    cv = slice((2 * p + h2) * DH, (2 * p + h2 + 1) * DH)
                    for g in range(NGRP):
                        gs = slice(g * 128, (g + 1) * 128)
                        nc.tensor.matmul(
                            av[h2 * 64:(h2 + 1) * 64, gs],
                            st["vt"][:, g, cv], st["pt"][p][h2][:, gs],
                            start=True, stop=True,
                            tile_position=(0, h2 * 64))
                # ao8 = fp8(av * rz16); SX folded via the 1/16 Z ones
                (nc.gpsimd if O["ao8_pool"] else nc.vector).tensor_tensor(
                    st["ao8"][:, p, :], av, st["rz"][p], ALU.mult)
                st["pt"][p] = None
                st["rz"][p] = None

            def proj_tile(st, j):
                ic = st["ic"]
                csl = slice(ic * CHUNK, (ic + 1) * CHUNK)
                xb = st["ln"][0]
                ps = ps_mm.tile([128, CHUNK], F32, tag="mm")
                cs = slice(j * 128, (j + 1) * 128)
                first = True
                terms = [(wo8_sb, st["ao8"])]
                if PJ_COMP:
                    terms += [(wor_sb, st["ao8"])]
                for ti, (w, a) in enumerate(terms):
                    for tp in range(KP):
                        ks = slice(2 * tp, 2 * tp + 2)
                        nc.tensor.matmul(
                            ps, w[:, ks, cs], a[:, ks, :],
                            start=first, stop=False,
                            perf_mode=DR)
                        first = False
                # residual: + (SX*SW*I) @ x_bf16, descaled on eviction
                nc.tensor.matmul(ps, idsc_sb, xb[:, j, :],
                                 start=False, stop=True,
                                 skip_group_check=True)
                o = opool.tile([128, CHUNK], F32, tag="o")
                nc.scalar.activation(o, ps, AF.Copy, scale=1.0 / (SX * SW))
                nc.sync.dma_start(out_v[:, j, csl], o)

            def new_state(ic):
                return {
                    "ic": ic,
                    "ln": None,
                    "n8": n8pool.tile([128, KT, CHUNK], F8, tag="n8",
                                      name=f"n8_{ic}"),
                    "nr8": n8pool.tile([128, KT, CHUNK], F8, tag="nr8",
                                       name=f"nr8_{ic}") if NEED_NR else None,
                    "qe": qallpool.tile([128, 2 * NPAIR, CHUNK], BF16,
                                        tag="qeall", name=f"qeall{ic}"),
                    "qp": qppool.tile([128, 2 * NPAIR, CHUNK], BF16,
                                      tag="qpall", name=f"qpall{ic}"),
                    "vt": vpool.tile([128, NGRP, DIM], BF16, tag="vtok",
                                     name=f"vtok{ic}"),
                    "ao8": aopool.tile([128, NPAIR, CHUNK], F8, tag="ao8",
                                       name=f"ao8_{ic}"),
                    "pt": [None] * NPAIR,
                    "rz": [None] * NPAIR,
                }

            # ---- software pipeline over chunks ----
            prev = None
            cur = new_state(0)
            cur["ln"] = ln_phase(0)
            ln_apply(cur)
            load_weights()
            for ic in range(NCHUNK):
                # QKV for cur; rotary per quarter once its perm is in
                for p in range(NPAIR):
                    qk_tile(cur, p)
                    qk_tile(cur, NPAIR + p)
                    if p % 4 == 3:
                        q0 = p - 3
                        perm_quarter(cur, slice(q0, q0 + 4))
                        perm_quarter(cur, slice(NPAIR + q0, NPAIR + q0 + 4))
                        for pp in range(q0, q0 + 4):
                            rotary(cur, pp)
                            rotary(cur, NPAIR + pp)
                if dbg and ic == 0:
                    nc.sync.dma_start(dbg_qe, cur["qe"])
                    nc.sync.dma_start(dbg_n8, cur["n8"])
                    if cur["nr8"] is not None:
                        nc.sync.dma_start(dbg_nr8, cur["nr8"])
                    nc.sync.dma_start(dbg_ab[:, 0, :], cur["ln"][1])
                    nc.sync.dma_start(dbg_ab[:, 1, :], cur["ln"][2])
                # V + attention S/softmax for cur
                for g in range(NGRP):
                    v_tile(cur, g)
                    attn_pair(cur, 2 * g)
                    attn_pair(cur, 2 * g + 1)
                if dbg and ic == 0:
                    nc.sync.dma_start(dbg_vt, cur["vt"])
                # LN (stats+rows+bcast) for next chunk overlaps AV/proj
                nxt = None
                if ic + 1 < NCHUNK:
                    nxt = new_state(ic + 1)
                    nxt["ln"] = ln_phase(ic + 1)
                # AV for cur interleaved with proj for prev and the
                # normalize/quantize of the next chunk
                for p in range(NPAIR):
                    attn_av(cur, p)
                    if prev is not None:
                        proj_tile(prev, p)
                    if nxt is not None:
                        ln_apply_tile(nxt, p)
                if dbg and ic == 0:
                    nc.sync.dma_start(dbg_ao, cur["ao8"])
                prev = cur
                cur = nxt

            # drain: proj for the last chunk
            for j in range(KT):
                proj_tile(prev, j)

    nc.compile()
    return nc


def _host_constants(w_qkv, w_out, gamma, beta):
    wg = (w_qkv.astype(np.float32) * gamma.astype(np.float32)[None, :])
    wqkvT = np.ascontiguousarray(wg.T).astype(np.float32)     # (1024, 3072)
    woutT = np.ascontiguousarray(w_out.astype(np.float32).T)  # (1024, 1024)

    def comp8(a):
        hi = (a * SW).astype(NPF8)
        lo = (a * SW - hi.astype(np.float32)).astype(NPF8)
        return hi, lo

    wqk8, wqkr = comp8(wqkvT[:, :2 * DIM])
    wv8, wvr = comp8(wqkvT[:, 2 * DIM:])
    wo8, wor = comp8(woutT)

    qkvbias = (w_qkv.astype(np.float32) @ beta.astype(np.float32)
               ).astype(np.float32)                            # (3072,)
    vbias = np.ascontiguousarray(
        np.broadcast_to(qkvbias[2 * DIM:].astype(NPBF16), (128, DIM)))

    inv_freq = (1.0 / (10000.0 ** (np.arange(0, DH, 2, dtype=np.float64)
                                   / DH))).astype(np.float64)  # (32,)
    p = np.arange(128)
    j = np.arange(CHUNK)
    pos = (j % WIN).astype(np.float64)
    freq = inv_freq[(p % DH) % 32]                             # (128,)
    ang = freq[:, None] * pos[None, :]                         # (128, 512)
    cosT = np.cos(ang).astype(NPBF16)
    sgn = np.where((p % DH) < 32, -1.0, 1.0)
    sinT = (sgn[:, None] * np.sin(ang)).astype(NPBF16)

    # rank-5 additive window mask: -MBIG off same-window blocks.
    # M[k, q] = -MBIG + MBIG * sum_w ind_w(k) ind_w(q)  (within each
    # 128-token group; the column pattern repeats per group)
    mskL = np.zeros((5, 128), np.float32)
    mskR = np.zeros((5, CHUNK), np.float32)
    mskL[0, :] = -20.0
    mskR[0, :] = MBIG / 20.0
    for w in range(4):
        mskL[1 + w, w * 32:(w + 1) * 32] = 20.0
        colw = (np.arange(CHUNK) % 128) // 32
        mskR[1 + w, :] = np.where(colw == w, MBIG / 20.0, 0.0)
    mskL = mskL.astype(NPBF16)
    mskR = mskR.astype(NPBF16)

    onesAB = np.zeros((128, 33, 2), NPBF16)
    onesAB[:, 0, 0] = 1.0
    onesAB[:, 32, 1] = 1.0
    # 1/SX so rz = recip(Z/16) = 16/Z bakes the fp8 scale into ao8
    ones64 = np.full((128, DH), 1.0 / SX, NPBF16)
    # SX folded into the LN broadcast so nrm16 = 16*normed
    onesrow = np.full((1, 128), SX, NPBF16)
    idscaled = (np.eye(128, dtype=np.float32) * (SX * SW)).astype(NPBF16)
    return dict(wqk8=wqk8, wqkr=wqkr, wv8=wv8, wvr=wvr, wo8=wo8, wor=wor,
                qkvbias=qkvbias, vbias=vbias, cosT=cosT, sinT=sinT,
                maskL=mskL, maskR=mskR, onesAB=onesAB, ones64=ones64,
                onesrow=onesrow, idscaled=idscaled)


def _run(inputs, trace=False, trace_cores=None, opts=None):
    x = np.asarray(inputs["x"], dtype=np.float32)
    consts = _host_constants(np.asarray(inputs["w_qkv"], np.float32),
                             np.asarray(inputs["w_out"], np.float32),
                             np.asarray(inputs["gamma"], np.float32),
                             np.asarray(inputs["beta"], np.float32))
    beta_nonzero = bool(np.any(np.asarray(inputs["beta"]) != 0))
    key = ("nc", beta_nonzero)
    if key not in _CACHE:
        _CACHE[key] = _build(beta_nonzero, opts)
    nc = _CACHE[key]

    in_maps = []
    for c in range(NCORES):
        m = dict(consts)
        m["x"] = np.ascontiguousarray(x[:, c * TLOC:(c + 1) * TLOC])
        if not beta_nonzero:
            m["vbias"] = np.zeros((128, DIM), NPBF16)
        in_maps.append(m)

    res = run_bass_kernel_spmd(nc, in_maps, list(range(NCORES)),
                               trace=trace,
                               trace_cores=trace_cores)
    out = np.concatenate([res.results[c]["out"] for c in range(NCORES)],
                         axis=1)
    return out, res


def kernel(**inputs):
    out, _ = _run(inputs)
    return out


# revision 35
# speedup vs baseline: 1.3518x; 1.1225x over previous
"""Trainium2 Bass kernel for local windowed MHA (nn_LocalMHA).

Computation (see reference): x (C=1024, T=16384) -> LayerNorm over C ->
QKV proj -> rotary (window-relative) -> per-head attention within windows
of 32 tokens -> out proj -> +x residual.

Sharding: T split across 8 cores (2048 tokens each); windows are local so
no communication is needed. Weights replicated.

v2 design (per core, per 512-token chunk):
  - x DMA-cast to bf16 (gpsimd dma); LN stats via ones-matmul; short row
    chain (Act Rsqrt) emitting bf16 rows; per-token a/b2 broadcast via K=1
    matmuls; ln_apply 2 DVE ops -> normed bf16; n8/nr8 fp8 pair (x8 + res)
    for error-compensated fp8 matmuls.
  - QKV q/k + v: fp8 e4m3 DoubleRow matmuls (K=256 per mm), 3-term
    compensation (x8*W8 + x8*Wr8 + r8*W8) accumulated in one PSUM group;
    eviction descales on ACT.  Out-proj: plain fp8 DoubleRow.
  - rotary on eviction: q_rot = q*cos + perm(q)*sin; muls on DVE, add on
    Pool; partition permutation via 4 SBUF->SBUF DMAs.
  - attention computed TRANSPOSED (S^T = k^T q per 128-token group):
    window mask added as a rank-5 K=5 matmul into the same PSUM; exp on
    ACT gives P^T directly in AV-ready layout (no stream transpose);
    Z via ones-matmul broadcast to both 64-row halves; one DVE reciprocal;
    AV matmul; normalization folded into the fp8 ao eviction (DVE stt).
  - out proj + residual (Pool add), DMA out.
"""

import numpy as np
import ml_dtypes

import concourse.bass as bass
import concourse.bacc as bacc
import concourse.tile as tile
import concourse.mybir as mybir
from concourse.bass_utils import run_bass_kernel_spmd

F32 = mybir.dt.float32
BF16 = mybir.dt.bfloat16
F8 = mybir.dt.float8e4
NPBF16 = ml_dtypes.bfloat16
NPF8 = ml_dtypes.float8_e4m3
AF = mybir.ActivationFunctionType
ALU = mybir.AluOpType
DR = mybir.MatmulPerfMode.DoubleRow

DIM = 1024
T = 16384
NCORES = 8
TLOC = T // NCORES          # 2048
CHUNK = 512
NCHUNK = TLOC // CHUNK      # 4
HEADS = 16
DH = 64
WIN = 32
NPAIR = HEADS // 2          # 8 head pairs <-> 128-row tiles
NGRP = CHUNK // 128         # 4 groups of 128 tokens (4 windows each)
KT = DIM // 128             # 8 k-tiles of the contraction dim
KP = KT // 2                # 4 DoubleRow k-pairs
EPS = 1e-5
SCALE = DH ** -0.5          # 0.125
SX = 16.0                   # fp8 scale for activations
SW = 1024.0                 # fp8 scale for weights
MBIG = 800.0                # additive mask in raw-logit units (=100 post
                            # scale); exp(-100+..) flushes to 0 in bf16

_CACHE = {}


def _build(beta_nonzero: bool, opts: dict | None = None):
    O = dict(xb_bufs=4, n8_bufs=2, normed_bufs=2, psmm_bufs=2, s_bufs=2,
             p_bufs=4, ao_bufs=2, misc_bufs=1, av_bufs=1,
             rotadd_pool="split", res_pool=True, nr8_pool=True,
             v_evict_dve=False, drain_evict_dve=False, ic0_sync_x=False,
             lnadd_pool=False, ao8_pool=False, xb_cast=True,
             ln_exp=False, x_prefetch=True,
             qk_mode="comp", v_mode="comp", proj_mode="fp8")
    if opts:
        O.update(opts)
    nc = bacc.Bacc("TRN2", target_bir_lowering=False, debug=False,
                   num_devices=NCORES)

    x_d = nc.dram_tensor("x", [DIM, TLOC], F32, kind="ExternalInput").ap()
    # fp8 weights: hi and residual ("comp" terms).  qk: 2048 out-ch,
    # v: 1024 out-ch, wo: 1024 out-ch.
    wqk8_d = nc.dram_tensor("wqk8", [DIM, 2 * DIM], F8,
                            kind="ExternalInput").ap()
    wqkr_d = nc.dram_tensor("wqkr", [DIM, 2 * DIM], F8,
                            kind="ExternalInput").ap()
    wv8_d = nc.dram_tensor("wv8", [DIM, DIM], F8, kind="ExternalInput").ap()
    wvr_d = nc.dram_tensor("wvr", [DIM, DIM], F8, kind="ExternalInput").ap()
    wo8_d = nc.dram_tensor("wo8", [DIM, DIM], F8, kind="ExternalInput").ap()
    wor_d = nc.dram_tensor("wor", [DIM, DIM], F8, kind="ExternalInput").ap()
    cos_d = nc.dram_tensor("cosT", [128, CHUNK], BF16,
                           kind="ExternalInput").ap()
    sin_d = nc.dram_tensor("sinT", [128, CHUNK], BF16,
                           kind="ExternalInput").ap()
    mskL_d = nc.dram_tensor("maskL", [5, 128], BF16,
                            kind="ExternalInput").ap()
    mskR_d = nc.dram_tensor("maskR", [5, CHUNK], BF16,
                            kind="ExternalInput").ap()
    ones_d = nc.dram_tensor("onesAB", [128, 33, 2], BF16,
                            kind="ExternalInput").ap()
    on64_d = nc.dram_tensor("ones64", [128, DH], BF16,
                            kind="ExternalInput").ap()
    oner_d = nc.dram_tensor("onesrow", [1, 128], BF16,
                            kind="ExternalInput").ap()
    idsc_d = nc.dram_tensor("idscaled", [128, 128], BF16,
                            kind="ExternalInput").ap()
    qb_d = nc.dram_tensor("qkvbias", [3 * DIM], F32, kind="ExternalInput").ap()
    vb_d = nc.dram_tensor("vbias", [128, DIM], BF16, kind="ExternalInput").ap()
    out_d = nc.dram_tensor("out", [DIM, TLOC], F32, kind="ExternalOutput").ap()
    dbg = O0 = (opts or {}).get("debug_dump", False)
    if dbg:
        dbg_qe = nc.dram_tensor("dbg_qe", [128, 16, CHUNK], BF16,
                                kind="ExternalOutput").ap()
        dbg_vt = nc.dram_tensor("dbg_vt", [128, NGRP, DIM], BF16,
                                kind="ExternalOutput").ap()
        dbg_pt = nc.dram_tensor("dbg_pt", [128, 2, CHUNK], BF16,
                                kind="ExternalOutput").ap()
        dbg_rz = nc.dram_tensor("dbg_rz", [128, CHUNK], BF16,
                                kind="ExternalOutput").ap()
        dbg_ao = nc.dram_tensor("dbg_ao", [128, NPAIR, CHUNK], F8,
                                kind="ExternalOutput").ap()
        dbg_n8 = nc.dram_tensor("dbg_n8", [128, KT, CHUNK], F8,
                                kind="ExternalOutput").ap()
        dbg_nr8 = nc.dram_tensor("dbg_nr8", [128, KT, CHUNK], F8,
                                 kind="ExternalOutput").ap()
        dbg_ab = nc.dram_tensor("dbg_ab", [128, 2, CHUNK], BF16,
                                kind="ExternalOutput").ap()

    x_v = x_d.rearrange("(t p) n -> p t n", p=128)        # (128, 8, 2048)
    wqk8_v = wqk8_d.rearrange("(t p) n -> p t n", p=128)  # (128, 8, 2048)
    wqkr_v = wqkr_d.rearrange("(t p) n -> p t n", p=128)
    wv8_v = wv8_d.rearrange("(t p) n -> p t n", p=128)    # (128, 8, 1024)
    wvr_v = wvr_d.rearrange("(t p) n -> p t n", p=128)
    wo8_v = wo8_d.rearrange("(t p) n -> p t n", p=128)
    wor_v = wor_d.rearrange("(t p) n -> p t n", p=128)
    qb_v = qb_d.rearrange("(t p) -> p t", p=128)          # (128, 24)
    out_v = out_d.rearrange("(t p) n -> p t n", p=128)    # (128, 8, 2048)

    QK_COMP = O["qk_mode"] == "comp"
    V_COMP = O["v_mode"] == "comp"
    PJ_COMP = O["proj_mode"] == "comp"
    NEED_NR = QK_COMP or V_COMP

    from contextlib import ExitStack

    with tile.TileContext(nc) as tc:
        with ExitStack() as stk:
            ec = stk.enter_context
            wpool = ec(tc.tile_pool(name="weights", bufs=1))
            cpool = ec(tc.tile_pool(name="consts", bufs=1))
            xpool = ec(tc.tile_pool(name="xin", bufs=O["xb_bufs"]))
            xsqpool = ec(tc.tile_pool(name="xsq", bufs=1))
            lnrow = ec(tc.tile_pool(name="lnrow", bufs=1))
            lntmp = ec(tc.tile_pool(name="lntmp", bufs=1))
            npool = ec(tc.tile_pool(name="normed", bufs=O["normed_bufs"]))
            n8pool = ec(tc.tile_pool(name="n8", bufs=O["n8_bufs"]))
            qallpool = ec(tc.tile_pool(name="qall", bufs=1))
            qppool = ec(tc.tile_pool(name="qperm", bufs=1))
            qepool = ec(tc.tile_pool(name="qkevict", bufs=2))
            vpool = ec(tc.tile_pool(name="vtok", bufs=1))
            ppool = ec(tc.tile_pool(name="attnP", bufs=O["p_bufs"]))
            zpool = ec(tc.tile_pool(name="attnZ", bufs=2))
            aopool = ec(tc.tile_pool(name="ao", bufs=O["ao_bufs"]))
            opool = ec(tc.tile_pool(name="outs", bufs=2))
            xtpool = ec(tc.tile_pool(name="xt", bufs=2))
            ps_mm = ec(tc.tile_pool(name="ps_mm", bufs=O["psmm_bufs"],
                                    space="PSUM"))
            ps_misc = ec(tc.tile_pool(name="ps_misc", bufs=O["misc_bufs"],
                                      space="PSUM"))
            ps_s = ec(tc.tile_pool(name="ps_s", bufs=O["s_bufs"],
                                   space="PSUM"))
            ps_av = ec(tc.tile_pool(name="ps_av", bufs=O["av_bufs"],
                                    space="PSUM"))
            ps_stats = ps_bc = ps_z = ps_misc

            # ---- constants ----
            cos_sb = cpool.tile([128, CHUNK], BF16, tag="cos")
            nc.sync.dma_start(cos_sb, cos_d)
            sin_sb = cpool.tile([128, CHUNK], BF16, tag="sin")
            nc.sync.dma_start(sin_sb, sin_d)
            mskL_sb = cpool.tile([5, 128], BF16, tag="mskL")
            nc.sync.dma_start(mskL_sb, mskL_d)
            mskR_sb = cpool.tile([5, CHUNK], BF16, tag="mskR")
            nc.sync.dma_start(mskR_sb, mskR_d)
            ones_sb = cpool.tile([128, 33, 2], BF16, tag="onesAB")
            nc.sync.dma_start(ones_sb, ones_d)
            on64_sb = cpool.tile([128, DH], BF16, tag="ones64")
            nc.sync.dma_start(on64_sb, on64_d)
            oner_sb = cpool.tile([1, 128], BF16, tag="onesrow")
            nc.sync.dma_start(oner_sb, oner_d)
            idsc_sb = cpool.tile([128, 128], BF16, tag="idscaled")
            nc.sync.dma_start(idsc_sb, idsc_d)
            qb_sb = cpool.tile([128, 24], F32, tag="qbias")
            nc.sync.dma_start(qb_sb, qb_v)
            vb_sb = None
            if beta_nonzero:
                vb_sb = cpool.tile([128, DIM], BF16, tag="vbias")
                nc.sync.dma_start(vb_sb, vb_d)
            eps_sb = cpool.tile([1, 1], F32, tag="eps")
            nc.vector.memset(eps_sb, EPS)

            wqk8_sb = wpool.tile([128, KT, 2 * DIM], F8, tag="wqk8")
            wqkr_sb = wpool.tile([128, KT, 2 * DIM], F8, tag="wqkr",
                                 name="wqkr_sb") if QK_COMP else None
            wv8_sb = wpool.tile([128, KT, DIM], F8, tag="wv8")
            wvr_sb = wpool.tile([128, KT, DIM], F8, tag="wvr",
                                name="wvr_sb") if V_COMP else None
            wo8_sb = wpool.tile([128, KT, DIM], F8, tag="wo8")
            wor_sb = wpool.tile([128, KT, DIM], F8, tag="wor",
                                name="wor_sb") if PJ_COMP else None

            def load_weights():
                # per-column-block so early qk tiles can start asap
                for jp in range(2 * NPAIR):
                    cs = slice(jp * 128, (jp + 1) * 128)
                    nc.sync.dma_start(wqk8_sb[:, :, cs], wqk8_v[:, :, cs])
                    if QK_COMP:
                        nc.sync.dma_start(wqkr_sb[:, :, cs], wqkr_v[:, :, cs])
                nc.sync.dma_start(wv8_sb, wv8_v)
                if V_COMP:
                    nc.sync.dma_start(wvr_sb, wvr_v)
                nc.sync.dma_start(wo8_sb, wo8_v)
                if PJ_COMP:
                    nc.sync.dma_start(wor_sb, wor_v)

            def ln_load(ic):
                """start the x DMAs for chunk ic early (they cast on the
                Pool engine and are slow)."""
                csl = slice(ic * CHUNK, (ic + 1) * CHUNK)
                xb = xpool.tile([128, KT, CHUNK], BF16, tag="xb",
                                name=f"xb{ic}")
                for t in range(KT):
                    nc.gpsimd.dma_start(xb[:, t, :], x_v[:, t, csl])
                return xb

            def ln_phase(ic, xb=None):
                """LN stats for chunk ic -> (xb, a_sb, b2_sb)."""
                if xb is None:
                    xb = ln_load(ic)
                stats = ps_stats.tile([33, CHUNK], F32, tag="stats")
                for t in range(KT):
                    nc.tensor.matmul(stats, ones_sb[:, :, 0], xb[:, t, :],
                                     start=(t == 0), stop=False)
                for t in range(KT):
                    xsq = xsqpool.tile([128, CHUNK], BF16, tag="xsq")
                    nc.vector.tensor_mul(xsq, xb[:, t, :], xb[:, t, :])
                    nc.tensor.matmul(stats, ones_sb[:, :, 1], xsq,
                                     start=False, stop=(t == KT - 1))

                mu = lnrow.tile([1, CHUNK], F32, tag="mu")
                nc.vector.tensor_scalar_mul(mu, stats[0:1, :], 1.0 / DIM)
                var = lnrow.tile([1, CHUNK], F32, tag="var")
                nc.vector.tensor_mul(var, mu, mu)
                nc.vector.scalar_tensor_tensor(var, stats[32:33, :],
                                               1.0 / DIM, var,
                                               ALU.mult, ALU.subtract)
                a_hi = lnrow.tile([1, CHUNK], BF16, tag="ahi")
                if O["ln_exp"]:
                    # rstd = exp(-0.5 ln(var+eps)): ln/exp share an ACT
                    # table (unlike Sqrt) so attention's Exp never swaps
                    nc.scalar.activation(var, var, AF.Ln, bias=eps_sb)
                    nc.scalar.activation(a_hi, var, AF.Exp, scale=-0.5)
                else:
                    nc.scalar.activation(var, var, AF.Sqrt, bias=eps_sb)
                    with nc.allow_low_precision(reason="rstd fits bf16"):
                        nc.vector.reciprocal(a_hi, var)
                b2_hi = lnrow.tile([1, CHUNK], BF16, tag="b2hi")
                nc.vector.scalar_tensor_tensor(b2_hi, mu, -1.0, a_hi,
                                               ALU.mult, ALU.mult)

                def bcast(hi, tag):
                    bc = ps_bc.tile([128, CHUNK], F32, tag="bc")
                    nc.tensor.matmul(bc, oner_sb, hi, start=True, stop=True)
                    sb = lntmp.tile([128, CHUNK], BF16, tag=tag + "sb",
                                    bufs=1)
                    nc.scalar.copy(sb, bc)
                    return sb

                a_sb = bcast(a_hi, "abc")
                b2_sb = bcast(b2_hi, "b2bc")
                return xb, a_sb, b2_sb

            def ln_apply_tile(st, t):
                """normalize x tile t, emit fp8 hi (+residual).

                a_sb/b2_sb carry a folded x16 so nrm16 = 16*normed; n8 is
                then a plain fp8 cast and nr8 a plain subtract.
                """
                xb, a_sb, b2_sb = st["ln"]
                tmp = lntmp.tile([128, CHUNK], BF16, tag="lntmp")
                nc.vector.tensor_mul(tmp, xb[:, t, :], a_sb)
                nrm = npool.tile([128, CHUNK], BF16, tag="nrm")
                (nc.gpsimd if O["lnadd_pool"] else nc.vector).tensor_add(
                    nrm, tmp, b2_sb)
                nc.scalar.copy(st["n8"][:, t, :], nrm)
                if NEED_NR:
                    (nc.gpsimd if O["nr8_pool"] else nc.vector).tensor_tensor(
                        st["nr8"][:, t, :], nrm, st["n8"][:, t, :],
                        ALU.subtract)

            def ln_apply(st):
                for t in range(KT):
                    ln_apply_tile(st, t)

            def qk_tile(st, jp):
                """project q/k tile jp of chunk st, evict raw to qe."""
                n8, nr8 = st["n8"], st["nr8"]
                ps = ps_mm.tile([128, CHUNK], F32, tag="mm")
                cs = slice(jp * 128, (jp + 1) * 128)
                first = True
                terms = [(wqk8_sb, n8)]
                if QK_COMP:
                    terms += [(wqkr_sb, n8), (wqk8_sb, nr8)]
                nterm = len(terms)
                for ti, (w, a) in enumerate(terms):
                    for tp in range(KP):
                        ks = slice(2 * tp, 2 * tp + 2)
                        nc.tensor.matmul(
                            ps, w[:, ks, cs], a[:, ks, :],
                            start=first,
                            stop=(ti == nterm - 1 and tp == KP - 1),
                            perf_mode=DR)
                        first = False
                nc.scalar.activation(st["qe"][:, jp, :], ps, AF.Identity,
                                     scale=1.0 / (SX * SW),
                                     bias=qb_sb[:, jp:jp + 1])

            def perm_quarter(st, hs):
                for a in range(4):
                    src = (a // 2) * 64 + ((a % 2) ^ 1) * 32
                    nc.sync.dma_start(
                        st["qp"][a * 32:(a + 1) * 32, hs, :],
                        st["qe"][src:src + 32, hs, :])

            def rotary(st, jp):
                t1 = qepool.tile([128, CHUNK], BF16, tag="rt1")
                nc.vector.tensor_mul(t1, st["qe"][:, jp, :], cos_sb)
                t2 = qepool.tile([128, CHUNK], BF16, tag="rt2")
                nc.vector.tensor_mul(t2, st["qp"][:, jp, :], sin_sb)
                ra = O["rotadd_pool"]
                if ra == "split":
                    eng = nc.gpsimd if jp % 2 else nc.vector
                else:
                    eng = nc.gpsimd if ra else nc.vector
                eng.tensor_add(st["qe"][:, jp, :], t1, t2)

            def v_tile(st, g):
                n8, nr8 = st["n8"], st["nr8"]
                gs = slice(g * 128, (g + 1) * 128)
                for hf in range(2):
                    ps = ps_mm.tile([128, CHUNK], F32, tag="mm")
                    ws = slice(hf * CHUNK, (hf + 1) * CHUNK)
                    first = True
                    terms = [(n8, wv8_sb)]
                    if V_COMP:
                        terms += [(n8, wvr_sb), (nr8, wv8_sb)]
                    nterm = len(terms)
                    for ti, (a, w) in enumerate(terms):
                        for tp in range(KP):
                            ks = slice(2 * tp, 2 * tp + 2)
                            nc.tensor.matmul(
                                ps, a[:, ks, gs], w[:, ks, ws],
                                start=first,
                                stop=(ti == nterm - 1 and tp == KP - 1),
                                perf_mode=DR)
                            first = False
                    vdst = st["vt"][:, g, ws]
                    if O["v_evict_dve"]:
                        nc.vector.tensor_scalar_mul(vdst, ps, 1.0 / (SX * SW))
                    else:
                        nc.scalar.activation(vdst, ps, AF.Copy,
                                             scale=1.0 / (SX * SW))
                    if beta_nonzero:
                        nc.vector.scalar_tensor_tensor(
                            vdst, vb_sb[:, ws], 1.0, vdst,
                            ALU.mult, ALU.add)

            def attn_pair(st, p):
                """S^T + exp + Z for pair p -> (pt tiles, rz)."""
                z_ps = ps_z.tile([128, CHUNK], F32, tag="z")
                pts = []
                for h2 in range(2):
                    rs = slice(h2 * 64, (h2 + 1) * 64)
                    s_ps = ps_s.tile([128, CHUNK], F32, tag="s")
                    # mask first: start=True zeroes the whole 2KB psum
                    # zero-region, so the group matmuls must accumulate
                    nc.tensor.matmul(s_ps, mskL_sb, mskR_sb,
                                     start=True, stop=False)
                    for g in range(NGRP):
                        gs = slice(g * 128, (g + 1) * 128)
                        nc.tensor.matmul(
                            s_ps[:, gs], st["qe"][rs, NPAIR + p, gs],
                            st["qe"][rs, p, gs],
                            start=False, stop=(g == NGRP - 1))
                    pt = ppool.tile([128, CHUNK], BF16, tag="pt")
                    nc.scalar.activation(pt, s_ps, AF.Exp, scale=SCALE)
                    nc.tensor.matmul(z_ps[rs, :], on64_sb, pt,
                                     start=True, stop=True,
                                     tile_position=(0, h2 * 64))
                    pts.append(pt)
                rz = zpool.tile([128, CHUNK], BF16, tag="rz")
                with nc.allow_low_precision(reason="1/Z fits bf16"):
                    nc.vector.reciprocal(rz, z_ps)
                if dbg and st["ic"] == 0 and p == 0:
                    for h2 in range(2):
                        nc.sync.dma_start(dbg_pt[:, h2, :], pts[h2])
                    nc.sync.dma_start(dbg_rz, rz)
                st["pt"][p] = pts
                st["rz"][p] = rz

            def attn_av(st, p):
                av = ps_av.tile([128, CHUNK], F32, tag="av")
                for h2 in range(2):
                    cv = slice((2 * p + h2) * DH, (2 * p + h2 + 1) * DH)
                    for g in range(NGRP):
                        gs = slice(g * 128, (g + 1) * 128)
                        nc.tensor.matmul(
                            av[h2 * 64:(h2 + 1) * 64, gs],
                            st["vt"][:, g, cv], st["pt"][p][h2][:, gs],
                            start=True, stop=True,
                            tile_position=(0, h2 * 64))
                # ao8 = fp8(av * rz16); SX folded via the 1/16 Z ones
                (nc.gpsimd if O["ao8_pool"] else nc.vector).tensor_tensor(
                    st["ao8"][:, p, :], av, st["rz"][p], ALU.mult)
                st["pt"][p] = None
                st["rz"][p] = None

            def proj_tile(st, j):
                ic = st["ic"]
                csl = slice(ic * CHUNK, (ic + 1) * CHUNK)
                xb = st["ln"][0]
                ps = ps_mm.tile([128, CHUNK], F32, tag="mm")
                cs = slice(j * 128, (j + 1) * 128)
                first = True
                terms = [(wo8_sb, st["ao8"])]
                if PJ_COMP:
                    terms += [(wor_sb, st["ao8"])]
                for ti, (w, a) in enumerate(terms):
                    for tp in range(KP):
                        ks = slice(2 * tp, 2 * tp + 2)
                        nc.tensor.matmul(
                            ps, w[:, ks, cs], a[:, ks, :],
                            start=first, stop=False,
                            perf_mode=DR)
                        first = False
                # residual: + (SX*SW*I) @ x_bf16, descaled on eviction
                nc.tensor.matmul(ps, idsc_sb, xb[:, j, :],
                                 start=False, stop=True,
                                 skip_group_check=True)
                o = opool.tile([128, CHUNK], F32, tag="o")
                if st.get("drain") and O["drain_evict_dve"]:
                    nc.vector.tensor_scalar_mul(o, ps, 1.0 / (SX * SW))
                else:
                    nc.scalar.activation(o, ps, AF.Copy, scale=1.0 / (SX * SW))
                nc.sync.dma_start(out_v[:, j, csl], o)

            def new_state(ic):
                return {
                    "ic": ic,
                    "ln": None,
                    "n8": n8pool.tile([128, KT, CHUNK], F8, tag="n8",
                                      name=f"n8_{ic}"),
                    "nr8": n8pool.tile([128, KT, CHUNK], F8, tag="nr8",
                                       name=f"nr8_{ic}") if NEED_NR else None,
                    "qe": qallpool.tile([128, 2 * NPAIR, CHUNK], BF16,
                                        tag="qeall", name=f"qeall{ic}"),
                    "qp": qppool.tile([128, 2 * NPAIR, CHUNK], BF16,
                                      tag="qpall", name=f"qpall{ic}"),
                    "vt": vpool.tile([128, NGRP, DIM], BF16, tag="vtok",
                                     name=f"vtok{ic}"),
                    "ao8": aopool.tile([128, NPAIR, CHUNK], F8, tag="ao8",
                                       name=f"ao8_{ic}"),
                    "pt": [None] * NPAIR,
                    "rz": [None] * NPAIR,
                }

            # ---- software pipeline over chunks ----
            prev = None
            cur = new_state(0)
            cur["ln"] = ln_phase(0)
            ln_apply(cur)
            load_weights()
            for ic in range(NCHUNK):
                # QKV for cur; rotary per quarter once its perm is in
                for p in range(NPAIR):
                    qk_tile(cur, p)
                    qk_tile(cur, NPAIR + p)
                    if p % 4 == 3:
                        q0 = p - 3
                        perm_quarter(cur, slice(q0, q0 + 4))
                        perm_quarter(cur, slice(NPAIR + q0, NPAIR + q0 + 4))
                        for pp in range(q0, q0 + 4):
                            rotary(cur, pp)
                            rotary(cur, NPAIR + pp)
                if dbg and ic == 0:
                    nc.sync.dma_start(dbg_qe, cur["qe"])
                    nc.sync.dma_start(dbg_n8, cur["n8"])
                    if cur["nr8"] is not None:
                        nc.sync.dma_start(dbg_nr8, cur["nr8"])
                    nc.sync.dma_start(dbg_ab[:, 0, :], cur["ln"][1])
                    nc.sync.dma_start(dbg_ab[:, 1, :], cur["ln"][2])
                xb_nxt = (ln_load(ic + 1)
                          if O["x_prefetch"] and ic + 1 < NCHUNK else None)
                # V for cur (PE/Act balanced)
                for g in range(NGRP):
                    v_tile(cur, g)
                if dbg and ic == 0:
                    nc.sync.dma_start(dbg_vt, cur["vt"])
                # LN (stats+rows+bcast) for next chunk
                nxt = None
                if ic + 1 < NCHUNK:
                    nxt = new_state(ic + 1)
                    nxt["ln"] = ln_phase(ic + 1, xb_nxt)
                # attention + proj(prev) + normalize(next), PE-bound
                for p in range(NPAIR):
                    attn_pair(cur, p)
                    attn_av(cur, p)
                    if prev is not None:
                        proj_tile(prev, p)
                    if nxt is not None:
                        ln_apply_tile(nxt, p)
                if dbg and ic == 0:
                    nc.sync.dma_start(dbg_ao, cur["ao8"])
                prev = cur
                cur = nxt

            # drain: proj for the last chunk
            prev["drain"] = True
            for j in range(KT):
                proj_tile(prev, j)

    nc.compile()
    return nc


def _host_constants(w_qkv, w_out, gamma, beta):
    wg = (w_qkv.astype(np.float32) * gamma.astype(np.float32)[None, :])
    wqkvT = np.ascontiguousarray(wg.T).astype(np.float32)     # (1024, 3072)
    woutT = np.ascontiguousarray(w_out.astype(np.float32).T)  # (1024, 1024)

    def comp8(a):
        hi = (a * SW).astype(NPF8)
        lo = (a * SW - hi.astype(np.float32)).astype(NPF8)
        return hi, lo

    wqk8, wqkr = comp8(wqkvT[:, :2 * DIM])
    wv8, wvr = comp8(wqkvT[:, 2 * DIM:])
    wo8, wor = comp8(woutT)

    qkvbias = (w_qkv.astype(np.float32) @ beta.astype(np.float32)
               ).astype(np.float32)                            # (3072,)
    vbias = np.ascontiguousarray(
        np.broadcast_to(qkvbias[2 * DIM:].astype(NPBF16), (128, DIM)))

    inv_freq = (1.0 / (10000.0 ** (np.arange(0, DH, 2, dtype=np.float64)
                                   / DH))).astype(np.float64)  # (32,)
    p = np.arange(128)
    j = np.arange(CHUNK)
    pos = (j % WIN).astype(np.float64)
    freq = inv_freq[(p % DH) % 32]                             # (128,)
    ang = freq[:, None] * pos[None, :]                         # (128, 512)
    cosT = np.cos(ang).astype(NPBF16)
    sgn = np.where((p % DH) < 32, -1.0, 1.0)
    sinT = (sgn[:, None] * np.sin(ang)).astype(NPBF16)

    # rank-5 additive window mask: -MBIG off same-window blocks.
    # M[k, q] = -MBIG + MBIG * sum_w ind_w(k) ind_w(q)  (within each
    # 128-token group; the column pattern repeats per group)
    mskL = np.zeros((5, 128), np.float32)
    mskR = np.zeros((5, CHUNK), np.float32)
    mskL[0, :] = -20.0
    mskR[0, :] = MBIG / 20.0
    for w in range(4):
        mskL[1 + w, w * 32:(w + 1) * 32] = 20.0
        colw = (np.arange(CHUNK) % 128) // 32
        mskR[1 + w, :] = np.where(colw == w, MBIG / 20.0, 0.0)
    mskL = mskL.astype(NPBF16)
    mskR = mskR.astype(NPBF16)

    onesAB = np.zeros((128, 33, 2), NPBF16)
    onesAB[:, 0, 0] = 1.0
    onesAB[:, 32, 1] = 1.0
    # 1/SX so rz = recip(Z/16) = 16/Z bakes the fp8 scale into ao8
    ones64 = np.full((128, DH), 1.0 / SX, NPBF16)
    # SX folded into the LN broadcast so nrm16 = 16*normed
    onesrow = np.full((1, 128), SX, NPBF16)
    idscaled = (np.eye(128, dtype=np.float32) * (SX * SW)).astype(NPBF16)
    return dict(wqk8=wqk8, wqkr=wqkr, wv8=wv8, wvr=wvr, wo8=wo8, wor=wor,
                qkvbias=qkvbias, vbias=vbias, cosT=cosT, sinT=sinT,
                maskL=mskL, maskR=mskR, onesAB=onesAB, ones64=ones64,
                onesrow=onesrow, idscaled=idscaled)


def _run(inputs, trace=False, trace_cores=None, opts=None):
    x = np.asarray(inputs["x"], dtype=np.float32)
    consts = _host_constants(np.asarray(inputs["w_qkv"], np.float32),
                             np.asarray(inputs["w_out"], np.float32),
                             np.asarray(inputs["gamma"], np.float32),
                             np.asarray(inputs["beta"], np.float32))
    beta_nonzero = bool(np.any(np.asarray(inputs["beta"]) != 0))
    key = ("nc", beta_nonzero)
    if key not in _CACHE:
        _CACHE[key] = _build(beta_nonzero, opts)
    nc = _CACHE[key]

    in_maps = []
    for c in range(NCORES):
        m = dict(consts)
        m["x"] = np.ascontiguousarray(x[:, c * TLOC:(c + 1) * TLOC])
        if not beta_nonzero:
            m["vbias"] = np.zeros((128, DIM), NPBF16)
        in_maps.append(m)

    res = run_bass_kernel_spmd(nc, in_maps, list(range(NCORES)),
                               trace=trace,
                               trace_cores=trace_cores)
    out = np.concatenate([res.results[c]["out"] for c in range(NCORES)],
                         axis=1)
    return out, res


def kernel(**inputs):
    out, _ = _run(inputs)
    return out


# revision 46
# speedup vs baseline: 1.4600x; 1.0800x over previous
"""Trainium2 Bass kernel for local windowed MHA (nn_LocalMHA).

Computation (see reference): x (C=1024, T=16384) -> LayerNorm over C ->
QKV proj -> rotary (window-relative) -> per-head attention within windows
of 32 tokens -> out proj -> +x residual.

Sharding: T split across 8 cores (2048 tokens each); windows are local so
no communication is needed. Weights replicated.

v2 design (per core, per 512-token chunk):
  - x DMA-cast to bf16 (gpsimd dma); LN stats via ones-matmul; short row
    chain (Act Rsqrt) emitting bf16 rows; per-token a/b2 broadcast via K=1
    matmuls; ln_apply 2 DVE ops -> normed bf16; n8/nr8 fp8 pair (x8 + res)
    for error-compensated fp8 matmuls.
  - QKV q/k + v: fp8 e4m3 DoubleRow matmuls (K=256 per mm), 3-term
    compensation (x8*W8 + x8*Wr8 + r8*W8) accumulated in one PSUM group;
    eviction descales on ACT.  Out-proj: plain fp8 DoubleRow.
  - rotary on eviction: q_rot = q*cos + perm(q)*sin; muls on DVE, add on
    Pool; partition permutation via 4 SBUF->SBUF DMAs.
  - attention computed TRANSPOSED (S^T = k^T q per 128-token group):
    window mask added as a rank-5 K=5 matmul into the same PSUM; exp on
    ACT gives P^T directly in AV-ready layout (no stream transpose);
    Z via ones-matmul broadcast to both 64-row halves; one DVE reciprocal;
    AV matmul; normalization folded into the fp8 ao eviction (DVE stt).
  - out proj + residual (Pool add), DMA out.
"""

import numpy as np
import ml_dtypes

import concourse.bass as bass
import concourse.bacc as bacc
import concourse.tile as tile
import concourse.mybir as mybir
from concourse.bass_utils import run_bass_kernel_spmd

F32 = mybir.dt.float32
BF16 = mybir.dt.bfloat16
F8 = mybir.dt.float8e4
NPBF16 = ml_dtypes.bfloat16
NPF8 = ml_dtypes.float8_e4m3
AF = mybir.ActivationFunctionType
ALU = mybir.AluOpType
DR = mybir.MatmulPerfMode.DoubleRow

DIM = 1024
T = 16384
NCORES = 8
TLOC = T // NCORES          # 2048
CHUNK = 512
NCHUNK = TLOC // CHUNK      # 4
HEADS = 16
DH = 64
WIN = 32
NPAIR = HEADS // 2          # 8 head pairs <-> 128-row tiles
NGRP = CHUNK // 128         # 4 groups of 128 tokens (4 windows each)
KT = DIM // 128             # 8 k-tiles of the contraction dim
KP = KT // 2                # 4 DoubleRow k-pairs
EPS = 1e-5
SCALE = DH ** -0.5          # 0.125
SX = 16.0                   # fp8 scale for activations
SW = 1024.0                 # fp8 scale for weights
MBIG = 800.0                # additive mask in raw-logit units (=100 post
                            # scale); exp(-100+..) flushes to 0 in bf16

_CACHE = {}


def _build(beta_nonzero: bool, opts: dict | None = None):
    O = dict(xb_bufs=4, n8_bufs=3, normed_bufs=4, psmm_bufs=2, s_bufs=2,
             p_bufs=6, ao_bufs=3, misc_bufs=1, av_bufs=1,
             rotadd_pool="split", res_pool=True, nr8_pool=True,
             v_evict_dve=False, drain_evict_dve=False, ic0_sync_x=False,
             drain_mod=2, cold_nr8_dve=False, xsq_prefetch=False, xsq_bufs=2,
             qe_bufs=3, o_bufs=3, lntmp_bufs=1,
             lnadd_pool=False, ao8_pool=False, xb_cast=True,
             ln_exp=False, x_prefetch=True,
             qk_mode="comp", v_mode="comp", proj_mode="fp8")
    if opts:
        O.update(opts)
    nc = bacc.Bacc("TRN2", target_bir_lowering=False, debug=False,
                   num_devices=NCORES)

    x_d = nc.dram_tensor("x", [DIM, TLOC], F32, kind="ExternalInput").ap()
    # fp8 weights: hi and residual ("comp" terms).  qk: 2048 out-ch,
    # v: 1024 out-ch, wo: 1024 out-ch.
    wqk8_d = nc.dram_tensor("wqk8", [DIM, 2 * DIM], F8,
                            kind="ExternalInput").ap()
    wqkr_d = nc.dram_tensor("wqkr", [DIM, 2 * DIM], F8,
                            kind="ExternalInput").ap()
    wv8_d = nc.dram_tensor("wv8", [DIM, DIM], F8, kind="ExternalInput").ap()
    wvr_d = nc.dram_tensor("wvr", [DIM, DIM], F8, kind="ExternalInput").ap()
    wo8_d = nc.dram_tensor("wo8", [DIM, DIM], F8, kind="ExternalInput").ap()
    wor_d = nc.dram_tensor("wor", [DIM, DIM], F8, kind="ExternalInput").ap()
    cos_d = nc.dram_tensor("cosT", [128, CHUNK], BF16,
                           kind="ExternalInput").ap()
    sin_d = nc.dram_tensor("sinT", [128, CHUNK], BF16,
                           kind="ExternalInput").ap()
    mskL_d = nc.dram_tensor("maskL", [5, 128], BF16,
                            kind="ExternalInput").ap()
    mskR_d = nc.dram_tensor("maskR", [5, CHUNK], BF16,
                            kind="ExternalInput").ap()
    ones_d = nc.dram_tensor("onesAB", [128, 33, 2], BF16,
                            kind="ExternalInput").ap()
    on64_d = nc.dram_tensor("ones64", [128, DH], BF16,
                            kind="ExternalInput").ap()
    oner_d = nc.dram_tensor("onesrow", [1, 128], BF16,
                            kind="ExternalInput").ap()
    idsc_d = nc.dram_tensor("idscaled", [128, 128], BF16,
                            kind="ExternalInput").ap()
    qb_d = nc.dram_tensor("qkvbias", [3 * DIM], F32, kind="ExternalInput").ap()
    vb_d = nc.dram_tensor("vbias", [128, DIM], BF16, kind="ExternalInput").ap()
    out_d = nc.dram_tensor("out", [DIM, TLOC], F32, kind="ExternalOutput").ap()
    dbg = O0 = (opts or {}).get("debug_dump", False)
    if dbg:
        dbg_qe = nc.dram_tensor("dbg_qe", [128, 16, CHUNK], BF16,
                                kind="ExternalOutput").ap()
        dbg_vt = nc.dram_tensor("dbg_vt", [128, NGRP, DIM], BF16,
                                kind="ExternalOutput").ap()
        dbg_pt = nc.dram_tensor("dbg_pt", [128, 2, CHUNK], BF16,
                                kind="ExternalOutput").ap()
        dbg_rz = nc.dram_tensor("dbg_rz", [128, CHUNK], BF16,
                                kind="ExternalOutput").ap()
        dbg_ao = nc.dram_tensor("dbg_ao", [128, NPAIR, CHUNK], F8,
                                kind="ExternalOutput").ap()
        dbg_n8 = nc.dram_tensor("dbg_n8", [128, KT, CHUNK], F8,
                                kind="ExternalOutput").ap()
        dbg_nr8 = nc.dram_tensor("dbg_nr8", [128, KT, CHUNK], F8,
                                 kind="ExternalOutput").ap()
        dbg_ab = nc.dram_tensor("dbg_ab", [128, 2, CHUNK], BF16,
                                kind="ExternalOutput").ap()

    x_v = x_d.rearrange("(t p) n -> p t n", p=128)        # (128, 8, 2048)
    wqk8_v = wqk8_d.rearrange("(t p) n -> p t n", p=128)  # (128, 8, 2048)
    wqkr_v = wqkr_d.rearrange("(t p) n -> p t n", p=128)
    wv8_v = wv8_d.rearrange("(t p) n -> p t n", p=128)    # (128, 8, 1024)
    wvr_v = wvr_d.rearrange("(t p) n -> p t n", p=128)
    wo8_v = wo8_d.rearrange("(t p) n -> p t n", p=128)
    wor_v = wor_d.rearrange("(t p) n -> p t n", p=128)
    qb_v = qb_d.rearrange("(t p) -> p t", p=128)          # (128, 24)
    out_v = out_d.rearrange("(t p) n -> p t n", p=128)    # (128, 8, 2048)

    QK_COMP = O["qk_mode"] == "comp"
    V_COMP = O["v_mode"] == "comp"
    PJ_COMP = O["proj_mode"] == "comp"
    NEED_NR = QK_COMP or V_COMP

    from contextlib import ExitStack

    with tile.TileContext(nc) as tc:
        with ExitStack() as stk:
            ec = stk.enter_context
            wpool = ec(tc.tile_pool(name="weights", bufs=1))
            cpool = ec(tc.tile_pool(name="consts", bufs=1))
            xpool = ec(tc.tile_pool(name="xin", bufs=4 * O["xb_bufs"]))
            xsqpool = ec(tc.tile_pool(name="xsq", bufs=O["xsq_bufs"]))
            lnrow = ec(tc.tile_pool(name="lnrow", bufs=1))
            lntmp = ec(tc.tile_pool(name="lntmp", bufs=O["lntmp_bufs"]))
            npool = ec(tc.tile_pool(name="normed", bufs=O["normed_bufs"]))
            n8pool = ec(tc.tile_pool(name="n8", bufs=4 * O["n8_bufs"]))
            qallpool = ec(tc.tile_pool(name="qall", bufs=1))
            qppool = ec(tc.tile_pool(name="qperm", bufs=1))
            qepool = ec(tc.tile_pool(name="qkevict", bufs=O["qe_bufs"]))
            vpool = ec(tc.tile_pool(name="vtok", bufs=1))
            ppool = ec(tc.tile_pool(name="attnP", bufs=O["p_bufs"]))
            zpool = ec(tc.tile_pool(name="attnZ", bufs=2))
            aopool = ec(tc.tile_pool(name="ao", bufs=O["ao_bufs"]))
            opool = ec(tc.tile_pool(name="outs", bufs=O["o_bufs"]))
            xtpool = ec(tc.tile_pool(name="xt", bufs=2))
            ps_mm = ec(tc.tile_pool(name="ps_mm", bufs=O["psmm_bufs"],
                                    space="PSUM"))
            ps_misc = ec(tc.tile_pool(name="ps_misc", bufs=O["misc_bufs"],
                                      space="PSUM"))
            ps_s = ec(tc.tile_pool(name="ps_s", bufs=O["s_bufs"],
                                   space="PSUM"))
            ps_av = ec(tc.tile_pool(name="ps_av", bufs=O["av_bufs"],
                                    space="PSUM"))
            ps_stats = ps_bc = ps_z = ps_misc

            # ---- constants ----
            cos_sb = cpool.tile([128, CHUNK], BF16, tag="cos")
            nc.sync.dma_start(cos_sb, cos_d)
            sin_sb = cpool.tile([128, CHUNK], BF16, tag="sin")
            nc.sync.dma_start(sin_sb, sin_d)
            mskL_sb = cpool.tile([5, 128], BF16, tag="mskL")
            nc.sync.dma_start(mskL_sb, mskL_d)
            mskR_sb = cpool.tile([5, CHUNK], BF16, tag="mskR")
            nc.sync.dma_start(mskR_sb, mskR_d)
            ones_sb = cpool.tile([128, 33, 2], BF16, tag="onesAB")
            nc.sync.dma_start(ones_sb, ones_d)
            on64_sb = cpool.tile([128, DH], BF16, tag="ones64")
            nc.sync.dma_start(on64_sb, on64_d)
            oner_sb = cpool.tile([1, 128], BF16, tag="onesrow")
            nc.sync.dma_start(oner_sb, oner_d)
            idsc_sb = cpool.tile([128, 128], BF16, tag="idscaled")
            nc.sync.dma_start(idsc_sb, idsc_d)
            qb_sb = cpool.tile([128, 24], F32, tag="qbias")
            nc.sync.dma_start(qb_sb, qb_v)
            vb_sb = None
            if beta_nonzero:
                vb_sb = cpool.tile([128, DIM], BF16, tag="vbias")
                nc.sync.dma_start(vb_sb, vb_d)
            eps_sb = cpool.tile([1, 1], F32, tag="eps")
            nc.vector.memset(eps_sb, EPS)

            wqk8_sb = wpool.tile([128, KT, 2 * DIM], F8, tag="wqk8")
            wqkr_sb = wpool.tile([128, KT, 2 * DIM], F8, tag="wqkr",
                                 name="wqkr_sb") if QK_COMP else None
            wv8_sb = wpool.tile([128, KT, DIM], F8, tag="wv8")
            wvr_sb = wpool.tile([128, KT, DIM], F8, tag="wvr",
                                name="wvr_sb") if V_COMP else None
            wo8_sb = wpool.tile([128, KT, DIM], F8, tag="wo8")
            wor_sb = wpool.tile([128, KT, DIM], F8, tag="wor",
                                name="wor_sb") if PJ_COMP else None

            def load_weights():
                # per-column-block so early qk tiles can start asap
                for jp in range(2 * NPAIR):
                    cs = slice(jp * 128, (jp + 1) * 128)
                    nc.sync.dma_start(wqk8_sb[:, :, cs], wqk8_v[:, :, cs])
                    if QK_COMP:
                        nc.sync.dma_start(wqkr_sb[:, :, cs], wqkr_v[:, :, cs])
                nc.sync.dma_start(wv8_sb, wv8_v)
                if V_COMP:
                    nc.sync.dma_start(wvr_sb, wvr_v)
                nc.sync.dma_start(wo8_sb, wo8_v)
                if PJ_COMP:
                    nc.sync.dma_start(wor_sb, wor_v)

            def ln_load(ic):
                """start the x DMAs for chunk ic early (they cast on the
                Pool engine and are slow).  One tile per DoubleRow k-pair
                so consumers wait only on their own slice."""
                csl = slice(ic * CHUNK, (ic + 1) * CHUNK)
                xb = [xpool.tile([128, 2, CHUNK], BF16, tag="xb",
                                 name=f"xb{ic}_{tp}") for tp in range(KP)]
                for t in range(KT):
                    nc.gpsimd.dma_start(xb[t // 2][:, t % 2, :],
                                        x_v[:, t, csl])
                return xb

            def ln_stats_tile(lns, t):
                """x and x^2 stats matmuls for k-tile t of a chunk."""
                xb_t = lns["xb"][t // 2][:, t % 2, :]
                nc.tensor.matmul(lns["stats"], ones_sb[:, :, 0], xb_t,
                                 start=(t == 0), stop=False)
                if lns["xsq"] is not None:
                    xsq = lns["xsq"][t]
                else:
                    xsq = xsqpool.tile([128, CHUNK], BF16, tag="xsq")
                    nc.vector.tensor_mul(xsq, xb_t, xb_t)
                nc.tensor.matmul(lns["stats"], ones_sb[:, :, 1], xsq,
                                 start=False, stop=(t == KT - 1))

            def ln_start(ic):
                lns = {"xb": ln_load(ic),
                       "stats": ps_stats.tile([33, CHUNK], F32,
                                              tag="stats",
                                              name=f"stats{ic}")}
                # x^2 up front: DVE is idle in the v phase and the casts
                # land progressively, so these never gate the stats mms
                if O["xsq_prefetch"]:
                    lns["xsq"] = []
                    for t in range(KT):
                        xb_t = lns["xb"][t // 2][:, t % 2, :]
                        xsq = xsqpool.tile([128, CHUNK], BF16, tag="xsq",
                                           name=f"xsq{ic}_{t}")
                        nc.vector.tensor_mul(xsq, xb_t, xb_t)
                        lns["xsq"].append(xsq)
                else:
                    lns["xsq"] = None
                return lns

            def ln_rows(lns):
                """row chain + broadcasts once all stats mms are in."""
                stats = lns["stats"]
                mu = lnrow.tile([1, CHUNK], F32, tag="mu")
                nc.vector.tensor_scalar_mul(mu, stats[0:1, :], 1.0 / DIM)
                var = lnrow.tile([1, CHUNK], F32, tag="var")
                nc.vector.tensor_mul(var, mu, mu)
                nc.vector.scalar_tensor_tensor(var, stats[32:33, :],
                                               1.0 / DIM, var,
                                               ALU.mult, ALU.subtract)
                a_hi = lnrow.tile([1, CHUNK], BF16, tag="ahi")
                if O["ln_exp"]:
                    # rstd = exp(-0.5 ln(var+eps)): ln/exp share an ACT
                    # table (unlike Sqrt) so attention's Exp never swaps
                    nc.scalar.activation(var, var, AF.Ln, bias=eps_sb)
                    nc.scalar.activation(a_hi, var, AF.Exp, scale=-0.5)
                else:
                    nc.scalar.activation(var, var, AF.Sqrt, bias=eps_sb)
                    with nc.allow_low_precision(reason="rstd fits bf16"):
                        nc.vector.reciprocal(a_hi, var)
                b2_hi = lnrow.tile([1, CHUNK], BF16, tag="b2hi")
                nc.vector.scalar_tensor_tensor(b2_hi, mu, -1.0, a_hi,
                                               ALU.mult, ALU.mult)

                def bcast(hi, tag):
                    bc = ps_bc.tile([128, CHUNK], F32, tag="bc")
                    nc.tensor.matmul(bc, oner_sb, hi, start=True, stop=True)
                    sb = lntmp.tile([128, CHUNK], BF16, tag=tag + "sb",
                                    bufs=1)
                    nc.scalar.copy(sb, bc)
                    return sb

                lns["a_sb"] = bcast(a_hi, "abc")
                lns["b2_sb"] = bcast(b2_hi, "b2bc")

            def ln_phase(ic):
                """cold-path LN: load + stats + rows in one go."""
                lns = ln_start(ic)
                for t in range(KT):
                    ln_stats_tile(lns, t)
                ln_rows(lns)
                return lns

            def ln_apply_tile(st, t, cold=False):
                """normalize x tile t, emit fp8 hi (+residual).

                a_sb/b2_sb carry a folded x16 so nrm16 = 16*normed; n8 is
                then a plain fp8 cast and nr8 a plain subtract.
                """
                ln = st["ln"]
                xb, a_sb, b2_sb = ln["xb"], ln["a_sb"], ln["b2_sb"]
                tmp = lntmp.tile([128, CHUNK], BF16, tag="lntmp")
                nc.vector.tensor_mul(tmp, xb[t // 2][:, t % 2, :], a_sb)
                nrm = npool.tile([128, CHUNK], BF16, tag="nrm")
                (nc.gpsimd if O["lnadd_pool"] else nc.vector).tensor_add(
                    nrm, tmp, b2_sb)
                n8d = st["n8"][t // 2][:, t % 2, :]
                nc.scalar.copy(n8d, nrm)
                if NEED_NR:
                    use_pool = O["nr8_pool"] and not (cold and
                                                      O["cold_nr8_dve"])
                    (nc.gpsimd if use_pool else nc.vector).tensor_tensor(
                        st["nr8"][t // 2][:, t % 2, :], nrm, n8d,
                        ALU.subtract)

            def ln_apply(st):
                for t in range(KT):
                    ln_apply_tile(st, t, cold=True)

            def qk_tile(st, jp):
                """project q/k tile jp of chunk st, evict raw to qe."""
                n8, nr8 = st["n8"], st["nr8"]
                ps = ps_mm.tile([128, CHUNK], F32, tag="mm")
                cs = slice(jp * 128, (jp + 1) * 128)
                first = True
                terms = [(wqk8_sb, n8)]
                if QK_COMP:
                    terms += [(wqkr_sb, n8), (wqk8_sb, nr8)]
                nterm = len(terms)
                for ti, (w, a) in enumerate(terms):
                    for tp in range(KP):
                        ks = slice(2 * tp, 2 * tp + 2)
                        nc.tensor.matmul(
                            ps, w[:, ks, cs], a[tp],
                            start=first,
                            stop=(ti == nterm - 1 and tp == KP - 1),
                            perf_mode=DR)
                        first = False
                nc.scalar.activation(st["qe"][:, jp, :], ps, AF.Identity,
                                     scale=1.0 / (SX * SW),
                                     bias=qb_sb[:, jp:jp + 1])

            def perm_quarter(st, hs):
                for a in range(4):
                    src = (a // 2) * 64 + ((a % 2) ^ 1) * 32
                    nc.sync.dma_start(
                        st["qp"][a * 32:(a + 1) * 32, hs, :],
                        st["qe"][src:src + 32, hs, :])

            def rotary(st, jp):
                t1 = qepool.tile([128, CHUNK], BF16, tag="rt1")
                nc.vector.tensor_mul(t1, st["qe"][:, jp, :], cos_sb)
                t2 = qepool.tile([128, CHUNK], BF16, tag="rt2")
                nc.vector.tensor_mul(t2, st["qp"][:, jp, :], sin_sb)
                ra = O["rotadd_pool"]
                if ra == "split":
                    eng = nc.gpsimd if jp % 2 else nc.vector
                else:
                    eng = nc.gpsimd if ra else nc.vector
                eng.tensor_add(st["qe"][:, jp, :], t1, t2)

            def v_tile(st, g):
                n8, nr8 = st["n8"], st["nr8"]
                gs = slice(g * 128, (g + 1) * 128)
                for hf in range(2):
                    ps = ps_mm.tile([128, CHUNK], F32, tag="mm")
                    ws = slice(hf * CHUNK, (hf + 1) * CHUNK)
                    first = True
                    terms = [(n8, wv8_sb)]
                    if V_COMP:
                        terms += [(n8, wvr_sb), (nr8, wv8_sb)]
                    nterm = len(terms)
                    for ti, (a, w) in enumerate(terms):
                        for tp in range(KP):
                            ks = slice(2 * tp, 2 * tp + 2)
                            nc.tensor.matmul(
                                ps, a[tp][:, :, gs], w[:, ks, ws],
                                start=first,
                                stop=(ti == nterm - 1 and tp == KP - 1),
                                perf_mode=DR)
                            first = False
                    vdst = st["vt"][:, g, ws]
                    if O["v_evict_dve"]:
                        nc.vector.tensor_scalar_mul(vdst, ps, 1.0 / (SX * SW))
                    else:
                        nc.scalar.activation(vdst, ps, AF.Copy,
                                             scale=1.0 / (SX * SW))
                    if beta_nonzero:
                        nc.vector.scalar_tensor_tensor(
                            vdst, vb_sb[:, ws], 1.0, vdst,
                            ALU.mult, ALU.add)

            def attn_pair(st, p):
                """S^T + exp + Z for pair p -> (pt tiles, rz)."""
                z_ps = ps_z.tile([128, CHUNK], F32, tag="z")
                pts = []
                for h2 in range(2):
                    rs = slice(h2 * 64, (h2 + 1) * 64)
                    s_ps = ps_s.tile([128, CHUNK], F32, tag="s")
                    # mask first: start=True zeroes the whole 2KB psum
                    # zero-region, so the group matmuls must accumulate
                    nc.tensor.matmul(s_ps, mskL_sb, mskR_sb,
                                     start=True, stop=False)
                    for g in range(NGRP):
                        gs = slice(g * 128, (g + 1) * 128)
                        nc.tensor.matmul(
                            s_ps[:, gs], st["qe"][rs, NPAIR + p, gs],
                            st["qe"][rs, p, gs],
                            start=False, stop=(g == NGRP - 1))
                    pt = ppool.tile([128, CHUNK], BF16, tag="pt")
                    nc.scalar.activation(pt, s_ps, AF.Exp, scale=SCALE)
                    nc.tensor.matmul(z_ps[rs, :], on64_sb, pt,
                                     start=True, stop=True,
                                     tile_position=(0, h2 * 64))
                    pts.append(pt)
                rz = zpool.tile([128, CHUNK], BF16, tag="rz")
                with nc.allow_low_precision(reason="1/Z fits bf16"):
                    nc.vector.reciprocal(rz, z_ps)
                if dbg and st["ic"] == 0 and p == 0:
                    for h2 in range(2):
                        nc.sync.dma_start(dbg_pt[:, h2, :], pts[h2])
                    nc.sync.dma_start(dbg_rz, rz)
                st["pt"][p] = pts
                st["rz"][p] = rz

            def attn_av(st, p):
                av = ps_av.tile([128, CHUNK], F32, tag="av")
                for h2 in range(2):
                    cv = slice((2 * p + h2) * DH, (2 * p + h2 + 1) * DH)
                    for g in range(NGRP):
                        gs = slice(g * 128, (g + 1) * 128)
                        nc.tensor.matmul(
                            av[h2 * 64:(h2 + 1) * 64, gs],
                            st["vt"][:, g, cv], st["pt"][p][h2][:, gs],
                            start=True, stop=True,
                            tile_position=(0, h2 * 64))
                # ao8 = fp8(av * rz16); SX folded via the 1/16 Z ones
                (nc.gpsimd if O["ao8_pool"] else nc.vector).tensor_tensor(
                    st["ao8"][:, p, :], av, st["rz"][p], ALU.mult)
                st["pt"][p] = None
                st["rz"][p] = None

            def proj_tile(st, j):
                ic = st["ic"]
                csl = slice(ic * CHUNK, (ic + 1) * CHUNK)
                xb = st["ln"]["xb"]
                ps = ps_mm.tile([128, CHUNK], F32, tag="mm")
                cs = slice(j * 128, (j + 1) * 128)
                first = True
                terms = [(wo8_sb, st["ao8"])]
                if PJ_COMP:
                    terms += [(wor_sb, st["ao8"])]
                for ti, (w, a) in enumerate(terms):
                    for tp in range(KP):
                        ks = slice(2 * tp, 2 * tp + 2)
                        nc.tensor.matmul(
                            ps, w[:, ks, cs], a[:, ks, :],
                            start=first, stop=False,
                            perf_mode=DR)
                        first = False
                # residual: + (SX*SW*I) @ x_bf16, descaled on eviction
                nc.tensor.matmul(ps, idsc_sb, xb[j // 2][:, j % 2, :],
                                 start=False, stop=True,
                                 skip_group_check=True)
                o = opool.tile([128, CHUNK], F32, tag="o")
                if st.get("drain") and j % 2 == O["drain_mod"]:
                    # alternate engines in the drain so the final
                    # evictions overlap instead of serializing on ACT
                    nc.vector.tensor_scalar_mul(o, ps, 1.0 / (SX * SW))
                else:
                    nc.scalar.activation(o, ps, AF.Copy, scale=1.0 / (SX * SW))
                nc.sync.dma_start(out_v[:, j, csl], o)

            def new_state(ic):
                return {
                    "ic": ic,
                    "ln": None,
                    "n8": [n8pool.tile([128, 2, CHUNK], F8, tag="n8",
                                       name=f"n8_{ic}_{tp}")
                           for tp in range(KP)],
                    "nr8": [n8pool.tile([128, 2, CHUNK], F8, tag="nr8",
                                        name=f"nr8_{ic}_{tp}")
                            for tp in range(KP)] if NEED_NR else None,
                    "qe": qallpool.tile([128, 2 * NPAIR, CHUNK], BF16,
                                        tag="qeall", name=f"qeall{ic}"),
                    "qp": qppool.tile([128, 2 * NPAIR, CHUNK], BF16,
                                      tag="qpall", name=f"qpall{ic}"),
                    "vt": vpool.tile([128, NGRP, DIM], BF16, tag="vtok",
                                     name=f"vtok{ic}"),
                    "ao8": aopool.tile([128, NPAIR, CHUNK], F8, tag="ao8",
                                       name=f"ao8_{ic}"),
                    "pt": [None] * NPAIR,
                    "rz": [None] * NPAIR,
                }

            # ---- software pipeline over chunks ----
            prev = None
            ln_states = {}
            ln_states[0] = ln_phase(0)
            cur = new_state(0)
            cur["ln"] = ln_states[0]
            ln_apply(cur)
            load_weights()
            ln_states[1] = ln_start(1)
            for ic in range(NCHUNK):
                # QKV for cur; rotary per quarter once its perm is in
                for p in range(NPAIR):
                    qk_tile(cur, p)
                    qk_tile(cur, NPAIR + p)
                    if p % 4 == 3:
                        q0 = p - 3
                        perm_quarter(cur, slice(q0, q0 + 4))
                        perm_quarter(cur, slice(NPAIR + q0, NPAIR + q0 + 4))
                        for pp in range(q0, q0 + 4):
                            rotary(cur, pp)
                            rotary(cur, NPAIR + pp)
                if dbg and ic == 0:
                    nc.sync.dma_start(dbg_qe, cur["qe"])
                    for tp in range(KP):
                        ks = slice(2 * tp, 2 * tp + 2)
                        nc.sync.dma_start(dbg_n8[:, ks, :], cur["n8"][tp])
                        if cur["nr8"] is not None:
                            nc.sync.dma_start(dbg_nr8[:, ks, :],
                                              cur["nr8"][tp])
                    nc.sync.dma_start(dbg_ab[:, 0, :], cur["ln"]["a_sb"])
                    nc.sync.dma_start(dbg_ab[:, 1, :], cur["ln"]["b2_sb"])
                # start x loads two chunks ahead (casts run on Pool
                # during the v/attention phases)
                if ic + 2 < NCHUNK:
                    ln_states[ic + 2] = ln_start(ic + 2)
                # V for cur (PE/Act balanced)
                for g in range(NGRP):
                    v_tile(cur, g)
                if dbg and ic == 0:
                    nc.sync.dma_start(dbg_vt, cur["vt"])
                nxt = None
                if ic + 1 < NCHUNK:
                    nxt = new_state(ic + 1)
                    if ic == 0:
                        # cold path: chunk 1 LN inline
                        for t in range(KT):
                            ln_stats_tile(ln_states[1], t)
                        ln_rows(ln_states[1])
                    nxt["ln"] = ln_states[ic + 1]
                # attention + proj(prev) + normalize(next) + stats(ic+2)
                for p in range(NPAIR):
                    attn_pair(cur, p)
                    attn_av(cur, p)
                    if prev is not None:
                        proj_tile(prev, p)
                    if nxt is not None:
                        ln_apply_tile(nxt, p)
                    if ic + 2 < NCHUNK:
                        ln_stats_tile(ln_states[ic + 2], p)
                # row chain for ic+2 overlaps the next qk phase
                if ic + 2 < NCHUNK:
                    ln_rows(ln_states[ic + 2])
                if dbg and ic == 0:
                    nc.sync.dma_start(dbg_ao, cur["ao8"])
                prev = cur
                cur = nxt

            # drain: proj for the last chunk
            prev["drain"] = True
            for j in range(KT):
                proj_tile(prev, j)

    nc.compile()
    return nc


def _host_constants(w_qkv, w_out, gamma, beta):
    wg = (w_qkv.astype(np.float32) * gamma.astype(np.float32)[None, :])
    wqkvT = np.ascontiguousarray(wg.T).astype(np.float32)     # (1024, 3072)
    woutT = np.ascontiguousarray(w_out.astype(np.float32).T)  # (1024, 1024)

    def comp8(a):
        hi = (a * SW).astype(NPF8)
        lo = (a * SW - hi.astype(np.float32)).astype(NPF8)
        return hi, lo

    wqk8, wqkr = comp8(wqkvT[:, :2 * DIM])
    wv8, wvr = comp8(wqkvT[:, 2 * DIM:])
    wo8, wor = comp8(woutT)

    qkvbias = (w_qkv.astype(np.float32) @ beta.astype(np.float32)
               ).astype(np.float32)                            # (3072,)
    vbias = np.ascontiguousarray(
        np.broadcast_to(qkvbias[2 * DIM:].astype(NPBF16), (128, DIM)))

    inv_freq = (1.0 / (10000.0 ** (np.arange(0, DH, 2, dtype=np.float64)
                                   / DH))).astype(np.float64)  # (32,)
    p = np.arange(128)
    j = np.arange(CHUNK)
    pos = (j % WIN).astype(np.float64)
    freq = inv_freq[(p % DH) % 32]                             # (128,)
    ang = freq[:, None] * pos[None, :]                         # (128, 512)
    cosT = np.cos(ang).astype(NPBF16)
    sgn = np.where((p % DH) < 32, -1.0, 1.0)
    sinT = (sgn[:, None] * np.sin(ang)).astype(NPBF16)

    # rank-5 additive window mask: -MBIG off same-window blocks.
    # M[k, q] = -MBIG + MBIG * sum_w ind_w(k) ind_w(q)  (within each
    # 128-token group; the column pattern repeats per group)
    mskL = np.zeros((5, 128), np.float32)
    mskR = np.zeros((5, CHUNK), np.float32)
    mskL[0, :] = -20.0
    mskR[0, :] = MBIG / 20.0
    for w in range(4):
        mskL[1 + w, w * 32:(w + 1) * 32] = 20.0
        colw = (np.arange(CHUNK) % 128) // 32
        mskR[1 + w, :] = np.where(colw == w, MBIG / 20.0, 0.0)
    mskL = mskL.astype(NPBF16)
    mskR = mskR.astype(NPBF16)

    onesAB = np.zeros((128, 33, 2), NPBF16)
    onesAB[:, 0, 0] = 1.0
    onesAB[:, 32, 1] = 1.0
    # 1/SX so rz = recip(Z/16) = 16/Z bakes the fp8 scale into ao8
    ones64 = np.full((128, DH), 1.0 / SX, NPBF16)
    # SX folded into the LN broadcast so nrm16 = 16*normed
    onesrow = np.full((1, 128), SX, NPBF16)
    idscaled = (np.eye(128, dtype=np.float32) * (SX * SW)).astype(NPBF16)
    return dict(wqk8=wqk8, wqkr=wqkr, wv8=wv8, wvr=wvr, wo8=wo8, wor=wor,
                qkvbias=qkvbias, vbias=vbias, cosT=cosT, sinT=sinT,
                maskL=mskL, maskR=mskR, onesAB=onesAB, ones64=ones64,
                onesrow=onesrow, idscaled=idscaled)


def _run(inputs, trace=False, trace_cores=None, opts=None):
    x = np.asarray(inputs["x"], dtype=np.float32)
    consts = _host_constants(np.asarray(inputs["w_qkv"], np.float32),
                             np.asarray(inputs["w_out"], np.float32),
                             np.asarray(inputs["gamma"], np.float32),
                             np.asarray(inputs["beta"], np.float32))
    beta_nonzero = bool(np.any(np.asarray(inputs["beta"]) != 0))
    key = ("nc", beta_nonzero)
    if key not in _CACHE:
        _CACHE[key] = _build(beta_nonzero, opts)
    nc = _CACHE[key]

    in_maps = []
    for c in range(NCORES):
        m = dict(consts)
        m["x"] = np.ascontiguousarray(x[:, c * TLOC:(c + 1) * TLOC])
        if not beta_nonzero:
            m["vbias"] = np.zeros((128, DIM), NPBF16)
        in_maps.append(m)

    res = run_bass_kernel_spmd(nc, in_maps, list(range(NCORES)),
                               trace=trace,
                               trace_cores=trace_cores)
    out = np.concatenate([res.results[c]["out"] for c in range(NCORES)],
                         axis=1)
    return out, res


def kernel(**inputs):
    out, _ = _run(inputs)
    return out
